# revision 35
# baseline (speedup 1.0000x reference)
"""GCN classifier (GCNConv + LayerNorm + ReLU + Linear) on 8 Trainium2 NeuronCores.

v2 strategy (self-contained; sized for N=100000, E=1600000, 128 ch, 16 classes):
  out = LN((A @ x) @ W1 + b1).relu() @ Wfc + bfc,  A = normalized adjacency.

  Profiling insights driving this design (vs v1 baseline @ 2.48ms):
  - SWDGE descriptor generation on the Q7 cores is ~8ns/descriptor and was
    2.0ms serial on one core pair. Fix: num_swdge_queues=4, one gather call
    per source bank on its own queue_num -> 4 Q7 pairs generate in parallel.
  - DVE tensor_scalar/copy can enter 2-port perf mode which takes an
    exclusive lock on the SBUF port shared with GPSIMD -> one-hot builds
    were blocking descriptor generation (and vice versa), 4.2ms of DVE time.
    Fix: build one-hot slabs with tensor_tensor (never contends) using
    stride-0 repeat APs; PSUM evacuations / scaling moved to the ACT engine
    (own SBUF port).
  - fp32 matmuls are 4 cycles/row on the PE; fp16 is 1. Everything on the
    matmul path is fp16 (tolerance is 2e-2; fp16 keeps us ~1e-3).
  - Padding trimmed: nodes are packed into 888 tiles so every (tile, bank)
    cell fits exactly K=4 chunks of 128 edges (~7% pad vs ~30%); self-loops
    are not gathered at all - they stream as dense 128-row blocks from a
    tile-permuted fp16 copy of x via HWDGE (free of Q7 descriptor cost).
  - LayerNorm mean comes free as an extra (negated row-mean) column of the
    W1 matmul; LN affine + ReLU fold into one ACT op in transposed layout.
"""
import heapq
import os

import numpy as np

N_NODES = 100000
IN_CH = 128
HIDDEN = 128
NUM_CLASSES = 16
LN_EPS = 1e-5
N_CORES = 8
P = 128
BANK = 25000
NBANK = 4
K = 4                 # chunks per (tile, bank)
CPT = NBANK * K + 1   # chunks per tile (16 gather + 1 dense self block)
CELLCAP = K * P       # max edges per (tile, bank)
TILES = 848
TPC = TILES // N_CORES
GS = 12               # tiles per gather group

LAST_RESULTS = None
_PROGRAM_CACHE = {}


def _groups():
    out = []
    t = 0
    while t < TPC:
        s = min(GS, TPC - t)
        out.append((s, t))
        t += s
    return out


def _call_col_bases():
    """Column base (in 16-wide int16 idx columns) of each (group, bank) gather."""
    bases = []
    run = 0
    for s, _ in _groups():
        row = []
        for _b in range(NBANK):
            row.append(run)
            run += s * K * P // 16
        bases.append(row)
    return bases, run


# ----------------------------------------------------------------------------
# host-side preprocessing
# ----------------------------------------------------------------------------
def _assign_tiles(dst, eb, cnt_nb):
    """LPT-pack nodes into TILES tiles (<=128 nodes each), then repair so every
    (tile, bank) cell holds <= CELLCAP edges."""
    N = N_NODES
    cnt = cnt_nb.sum(axis=1)
    order = np.argsort(-cnt, kind="stable")
    heap = [(0, t) for t in range(TILES)]
    heapq.heapify(heap)
    node_cnt = np.zeros(TILES, dtype=np.int64)
    edge_sum = np.zeros(TILES, dtype=np.int64)
    node_tile = np.empty(N, dtype=np.int64)
    for nd in order:
        while True:
            s, t = heapq.heappop(heap)
            if node_cnt[t] < P:
                break
        node_tile[nd] = t
        node_cnt[t] += 1
        edge_sum[t] += cnt[nd]
        if node_cnt[t] < P:
            heapq.heappush(heap, (edge_sum[t], t))

    # repair per-bank overflows
    for _ in range(64):
        cell = np.zeros((TILES, NBANK), dtype=np.int64)
        np.add.at(cell, (node_tile[dst], eb), 1)
        over = np.argwhere(cell > CELLCAP)
        if len(over) == 0:
            break
        node_cnt = np.bincount(node_tile, minlength=TILES)
        for t, b in over:
            excess = cell[t, b] - CELLCAP
            if excess <= 0:
                continue
            nodes_t = np.where(node_tile == t)[0]
            cand = nodes_t[np.argsort(-cnt_nb[nodes_t, b], kind="stable")]
            for nd in cand:
                if excess <= 0:
                    break
                c_nd = cnt_nb[nd]
                if c_nd[b] == 0:
                    break
                ok = (node_cnt < P) & ((cell + c_nd[None, :]) <= CELLCAP).all(axis=1)
                ok[t] = False
                if not ok.any():
                    continue
                cand_t2 = np.where(ok)[0]
                t2 = cand_t2[np.argmin(cell[cand_t2].sum(axis=1))]
                node_tile[nd] = t2
                cell[t] -= c_nd
                cell[t2] += c_nd
                node_cnt[t] -= 1
                node_cnt[t2] += 1
                excess = cell[t, b] - CELLCAP
    else:
        raise RuntimeError("tile repair did not converge")

    # compact slots within each tile
    order2 = np.argsort(node_tile, kind="stable")
    tile_sorted = node_tile[order2]
    starts = np.zeros(TILES + 1, dtype=np.int64)
    np.cumsum(np.bincount(tile_sorted, minlength=TILES), out=starts[1:])
    node_slot = np.empty(N, dtype=np.int64)
    node_slot[order2] = np.arange(N) - starts[tile_sorted]
    assert (node_slot < P).all()
    return node_tile, node_slot


def _preprocess(edge_index, edge_weight):
    src = np.asarray(edge_index[0], dtype=np.int64)
    dst = np.asarray(edge_index[1], dtype=np.int64)
    w = np.asarray(edge_weight, dtype=np.float32)
    N = N_NODES

    deg = np.bincount(dst, weights=w.astype(np.float64), minlength=N) + 1.0
    dinv = (1.0 / np.sqrt(deg)).astype(np.float32)
    norm = (dinv[src] * w * dinv[dst]).astype(np.float32)
    selfnorm = (dinv.astype(np.float64) ** 2).astype(np.float32)  # 1/deg

    eb = src // BANK
    cnt_nb = np.zeros((N, NBANK), dtype=np.int64)
    np.add.at(cnt_nb, (dst, eb), 1)
    node_tile, node_slot = _assign_tiles(dst, eb, cnt_nb)

    # per-edge (tile, bank) cell position
    et = node_tile[dst]
    keys = et * NBANK + eb
    eorder = np.argsort(keys, kind="stable")
    keys_s = keys[eorder]
    cum = np.zeros(TILES * NBANK + 1, dtype=np.int64)
    np.cumsum(np.bincount(keys_s, minlength=TILES * NBANK), out=cum[1:])
    pos = np.arange(len(keys_s)) - cum[keys_s]
    kk = pos // P
    lane = pos % P
    assert (kk < K).all(), "cell overflow after repair"

    src_s = src[eorder]
    dst_s = dst[eorder]
    et_s = et[eorder]
    eb_s = eb[eorder]
    norm_s = norm[eorder]

    # tile-major metadata, duplicated-pair layout [128, tile*(16*2) + (bank*K+kk)*2 + {0,1}]
    # (pairs give every DVE operand an innermost stride-1 dim -> 2x perf mode)
    GCH = NBANK * K  # gathered chunks per tile (self handled via selfoh)
    MCOLS = TILES * GCH * 2
    mcol = (et_s * GCH + eb_s * K + kk) * 2
    dstl_all = np.zeros((P, MCOLS), dtype=np.float16)
    norm_all = np.zeros((P, MCOLS), dtype=np.float16)
    dstl_all[lane, mcol] = node_slot[dst_s].astype(np.float16)
    dstl_all[lane, mcol + 1] = dstl_all[lane, mcol]
    norm_all[lane, mcol] = norm_s.astype(np.float16)
    norm_all[lane, mcol + 1] = norm_all[lane, mcol]

    # self-loop contribution folded post-aggregation: transposed, pre-scaled
    # x rows in tile-permuted order; added into Ps [ch, d] by one DVE op/tile
    perm_rows = node_tile * P + node_slot
    selfnorm16 = selfnorm.astype(np.float16)

    # gather indices, call-major: per core, per (group, bank) call,
    # within call linear i = (j*K + kk)*128 + lane
    bases, IDXC = _call_col_bases()
    core = et_s // TPC
    tl = et_s % TPC
    g = tl // GS
    j = tl % GS
    cb = np.asarray([[bases[gi][bi] for bi in range(NBANK)]
                     for gi in range(len(bases))], dtype=np.int64)
    i_lin = (j * K + kk) * P + lane
    col16 = core * IDXC + cb[g, eb_s] + i_lin // 16
    row16 = i_lin % 16
    idx16 = np.zeros((16, N_CORES * IDXC), dtype=np.int16)
    idx16[row16, col16] = (src_s % BANK).astype(np.int16)
    idx_all = np.tile(idx16, (8, 1))

    return dict(
        idx_all=idx_all, norm_all=norm_all, dstl_all=dstl_all,
        perm_rows=perm_rows, selfnorm16=selfnorm16,
        node_tile=node_tile, node_slot=node_slot, IDXC=IDXC,
    )


# ----------------------------------------------------------------------------
# device program
# ----------------------------------------------------------------------------
def _build_program():
    from contextlib import ExitStack
    import concourse.bass as bass
    import concourse.tile as tile
    from concourse import bacc, mybir

    f32 = mybir.dt.float32
    f16 = mybir.dt.float16
    i16 = mybir.dt.int16
    H = HIDDEN
    GCH = NBANK * K
    MC = TPC * GCH * 2
    bases, IDXC = _call_col_bases()

    nc = bacc.Bacc("TRN2", target_bir_lowering=False, debug=False,
                   num_devices=N_CORES, num_swdge_queues=4)
    xb = [nc.dram_tensor(f"xb{b}", [BANK, IN_CH], f16, kind="ExternalInput").ap()
          for b in range(NBANK)]
    xts_d = nc.dram_tensor("xts", [P, TPC * P], f16, kind="ExternalInput").ap()
    idx_d = nc.dram_tensor("idx", [P, IDXC], i16, kind="ExternalInput").ap()
    dstl_d = nc.dram_tensor("dstl", [P, MC], f16, kind="ExternalInput").ap()
    norm_d = nc.dram_tensor("normv", [P, MC], f16, kind="ExternalInput").ap()
    w1_d = nc.dram_tensor("W1aug", [IN_CH, H + 1], f16, kind="ExternalInput").ap()
    wfc_d = nc.dram_tensor("Wfc", [H, NUM_CLASSES], f16, kind="ExternalInput").ap()
    c1_d = nc.dram_tensor("c1", [1, H], f32, kind="ExternalInput").ap()
    lng_d = nc.dram_tensor("ln_g", [H, 1], f32, kind="ExternalInput").ap()
    lnb_d = nc.dram_tensor("ln_b", [H, 1], f32, kind="ExternalInput").ap()
    bfc_d = nc.dram_tensor("bfc", [1, NUM_CLASSES], f32, kind="ExternalInput").ap()
    iota_d = nc.dram_tensor("iota", [1, P], f16, kind="ExternalInput").ap()
    idm_d = nc.dram_tensor("idm", [P, P], f16, kind="ExternalInput").ap()
    out_d = nc.dram_tensor("out", [TPC * P, NUM_CLASSES], f32,
                           kind="ExternalOutput").ap()

    def bcast(src_ap, parts=P):
        return bass.AP(tensor=src_ap.tensor, offset=src_ap.offset,
                       ap=[[0, parts]] + list(src_ap.ap[1:]))

    def rep_mid(ap2d, n):
        """[p, q] -> [p, n, q] with the middle dim broadcast (stride 0)."""
        a = list(ap2d.ap)
        return bass.AP(tensor=ap2d.tensor, offset=ap2d.offset,
                       ap=[a[0], [0, n], a[1]])

    # 4D APs for the one-hot slab build; every operand keeps an innermost
    # stride-1 dim of size 2 so the DVE can enter 2x_1P perf mode.
    def meta_rep4(ap2d):
        """paired meta [p, 2*GCH] -> [p, GCH, 64, 2]; value const along dim 64."""
        a = list(ap2d.ap)
        return bass.AP(tensor=ap2d.tensor, offset=ap2d.offset,
                       ap=[a[0], [2, GCH], [0, P // 2], [1, 2]])

    def iota_rep4(ap2d):
        """IOTA [p, 128] -> [p, GCH, 64, 2]; iota along the last two dims."""
        a = list(ap2d.ap)
        return bass.AP(tensor=ap2d.tensor, offset=ap2d.offset,
                       ap=[a[0], [0, GCH], [2, P // 2], [1, 2]])

    def slab4(ap2d):
        """slab [p, GCH*128] -> [p, GCH, 64, 2] contiguous."""
        a = list(ap2d.ap)
        return bass.AP(tensor=ap2d.tensor, offset=ap2d.offset,
                       ap=[a[0], [P, GCH], [2, P // 2], [1, 2]])

    AL = mybir.AluOpType
    AF = mybir.ActivationFunctionType

    with tile.TileContext(nc) as tc, ExitStack() as ctx:
        consts = ctx.enter_context(tc.tile_pool(name="consts", bufs=1))
        gpool = ctx.enter_context(tc.tile_pool(name="gather", bufs=2))
        ohp = ctx.enter_context(tc.tile_pool(name="onehot", bufs=2))
        sp = ctx.enter_context(tc.tile_pool(name="work", bufs=4))
        t1p = ctx.enter_context(tc.tile_pool(name="t1c", bufs=2 * GS))
        statp = ctx.enter_context(tc.tile_pool(name="stats", bufs=4 * GS))
        pp_ps = ctx.enter_context(tc.tile_pool(name="pp_ps", bufs=2, space="PSUM"))
        agg_ps = ctx.enter_context(tc.tile_pool(name="agg_ps", bufs=2, space="PSUM"))
        tr_ps = ctx.enter_context(tc.tile_pool(name="tr_ps", bufs=2, space="PSUM"))
        fc_ps = ctx.enter_context(tc.tile_pool(name="fc_ps", bufs=2, space="PSUM"))

        W1s = consts.tile([IN_CH, H + 1], f16)
        nc.sync.dma_start(W1s[:], w1_d[:])
        Wfcs = consts.tile([H, NUM_CLASSES], f16)
        nc.sync.dma_start(Wfcs[:], wfc_d[:])
        C1s = consts.tile([P, H], f32)
        nc.sync.dma_start(C1s[:], bcast(c1_d))
        Gcol = consts.tile([H, 1], f32)
        nc.sync.dma_start(Gcol[:], lng_d[:])
        Bcol = consts.tile([H, 1], f32)
        nc.sync.dma_start(Bcol[:], lnb_d[:])
        BFCs = consts.tile([P, NUM_CLASSES], f32)
        nc.sync.dma_start(BFCs[:], bcast(bfc_d))
        IOTA = consts.tile([P, P], f16)
        nc.sync.dma_start(IOTA[:], bcast(iota_d))
        idents = consts.tile([P, P], f16)
        nc.sync.dma_start(idents[:], idm_d[:])
        eps_t = consts.tile([P, 1], f32)
        nc.vector.memset(eps_t[:], LN_EPS)

        # metadata tiles loaded in per-group slices inside the loop so group 0
        # starts as soon as its own slice lands
        idx_s = consts.tile([P, IDXC], i16)
        dstl_s = consts.tile([P, MC], f16)
        norm_s = consts.tile([P, MC], f16)

        out_acc = consts.tile([P, TPC * NUM_CLASSES], f32)

        # Software-pipelined emission: the PE stream is kept dense by skewing
        # every cross-engine round trip behind enough chunk-matmul work that
        # its dependencies are already resolved when the PE (or DVE/ACT FIFO
        # head) reaches it.
        def emit_add(st):
            # fold self-loop contribution: Ps2 = Ps + (selfnorm-scaled x^T)
            Ps2 = sp.tile([IN_CH, P], f16, tag="Ps2")
            j = st["j"]
            nc.vector.tensor_tensor(
                out=Ps2[:], in0=st["Ps"][:],
                in1=st["xtg"][:, j * P:(j + 1) * P], op=AL.add)
            st["Ps2"] = Ps2

        def emit_agg(st):
            agg = agg_ps.tile([P, H + 1], f32, space="PSUM")
            nc.tensor.matmul(agg[:], lhsT=st["Ps2"][:], rhs=W1s[:],
                             start=True, stop=True)
            st["agg"] = agg

        def emit_stats(st):
            agg = st["agg"]
            mu = statp.tile([P, 1], f32, tag="mu")
            nc.scalar.activation(out=mu[:], in_=agg[:, H:H + 1], func=AF.Copy)
            t1c = t1p.tile([P, H], f32, tag="t1c")
            nc.vector.scalar_tensor_tensor(
                out=t1c[:], in0=agg[:, 0:H], scalar=mu[:], in1=C1s[:],
                op0=AL.add, op1=AL.add)
            sq = sp.tile([P, H], f32, tag="sq")
            nc.scalar.activation(out=sq[:], in_=t1c[:], func=AF.Square,
                                 accum_out=st["vars"][:, st["j"]:st["j"] + 1])
            st["t1c"] = t1c

        def emit_b1(st):
            t1n = sp.tile([P, H], f16, tag="t1n")
            nc.scalar.activation(out=t1n[:], in_=st["t1c"][:], func=AF.Copy,
                                 scale=st["rstd"][:, st["j"]:st["j"] + 1])
            yT = tr_ps.tile([H, P], f16, space="PSUM")
            nc.tensor.transpose(out=yT[:], in_=t1n[:], identity=idents[:])
            st["yT"] = yT

        def emit_b2(st):
            hrT = sp.tile([H, P], f16, tag="hrT")
            nc.scalar.activation(out=hrT[:], in_=st["yT"][:], func=AF.Relu,
                                 scale=Gcol[:], bias=Bcol[:])
            j = st["j"]
            nc.tensor.matmul(st["fc"][:, j * NUM_CLASSES:(j + 1) * NUM_CLASSES],
                             lhsT=hrT[:], rhs=Wfcs[:], start=True, stop=True)

        def emit_outadd(gst):
            s, tb, fc = gst["s"], gst["tb"], gst["fc"]
            oslice = out_acc[:, tb * NUM_CLASSES:(tb + s) * NUM_CLASSES]
            o3 = oslice.rearrange("p (t c) -> p t c", c=NUM_CLASSES)
            f3 = fc[:, 0:s * NUM_CLASSES].rearrange("p (t c) -> p t c",
                                                    c=NUM_CLASSES)
            nc.vector.tensor_tensor(out=o3, in0=f3,
                                    in1=rep_mid(BFCs[:], s), op=AL.add)

        prev_states = None   # tile states of the previous group (pass B pending)
        prev_gst = None
        for gi, (s, tb) in enumerate(_groups()):
            ic0 = bases[gi][0]
            ic1 = bases[gi][NBANK - 1] + s * K * P // 16
            nc.sync.dma_start(idx_s[:, ic0:ic1], idx_d[:, ic0:ic1])
            mc_lo = tb * GCH * 2
            mc_hi = (tb + s) * GCH * 2
            nc.sync.dma_start(dstl_s[:, mc_lo:mc_hi], dstl_d[:, mc_lo:mc_hi])
            nc.sync.dma_start(norm_s[:, mc_lo:mc_hi], norm_d[:, mc_lo:mc_hi])

            Gg = gpool.tile([P, s * GCH, IN_CH], f16, tag="Gg")
            for b in range(NBANK):
                n = s * K * P
                cbase = bases[gi][b]
                nc.gpsimd.dma_gather(
                    out_ap=Gg[:, b * s * K:(b + 1) * s * K, :],
                    in_ap=xb[b][:],
                    idxs_ap=idx_s[:, cbase:cbase + n // 16],
                    num_idxs=n, num_idxs_reg=n, elem_size=IN_CH,
                    single_packet=False, queue_num=b,
                )
            # self-loop columns (pre-scaled x^T, tile-permuted) via HWDGE
            xtg = gpool.tile([P, s * P], f16, tag="xtg")
            xt_in = bass.AP(tensor=xts_d.tensor,
                            offset=xts_d.offset + tb * P,
                            ap=[[TPC * P, P], [1, s * P]])
            nc.sync.dma_start(xtg[:], xt_in)

            vars_g = statp.tile([P, s], f32, tag="vars")
            states = []
            for j in range(s):
                t = tb + j
                mc0 = t * GCH * 2
                dsl = dstl_s[:, mc0:mc0 + GCH * 2]
                nsl = norm_s[:, mc0:mc0 + GCH * 2]
                tmp = ohp.tile([P, GCH * P], f16, tag="tmp")
                nc.vector.tensor_tensor(out=slab4(tmp[:]), in0=meta_rep4(dsl),
                                        in1=iota_rep4(IOTA[:]), op=AL.is_equal)
                ohs = ohp.tile([P, GCH * P], f16, tag="ohs")
                nc.vector.tensor_tensor(out=slab4(ohs[:]), in0=slab4(tmp[:]),
                                        in1=meta_rep4(nsl), op=AL.mult)

                Pp = pp_ps.tile([IN_CH, P], f32, space="PSUM")
                for c in range(GCH):
                    gcol = (c // K) * s * K + j * K + (c % K)
                    nc.tensor.matmul(Pp[:], lhsT=Gg[:, gcol, :],
                                     rhs=ohs[:, c * P:(c + 1) * P],
                                     start=(c == 0), stop=(c == GCH - 1))
                Ps = sp.tile([IN_CH, P], f16, tag="Ps")
                nc.scalar.activation(out=Ps[:], in_=Pp[:], func=AF.Copy)
                states.append({"j": j, "Ps": Ps, "vars": vars_g, "xtg": xtg})

                # skewed tails: add+agg one tile behind, stats two tiles behind
                if j >= 1:
                    emit_add(states[j - 1])
                    emit_agg(states[j - 1])
                if j >= 2:
                    emit_stats(states[j - 2])
                # previous group's pass B: b1 at tile j, b2 one tile later
                if prev_states is not None:
                    if j < len(prev_states):
                        emit_b1(prev_states[j])
                    if 1 <= j <= len(prev_states):
                        emit_b2(prev_states[j - 1])

            # group-boundary flush of pass A
            emit_add(states[s - 1])
            emit_agg(states[s - 1])
            for jj in range(max(0, s - 2), s):
                emit_stats(states[jj])
            if prev_states is not None:
                for k in range(s, len(prev_states)):
                    emit_b1(prev_states[k])
                for k in range(max(1, s), len(prev_states) + 1):
                    emit_b2(prev_states[k - 1])
                emit_outadd(prev_gst)

            stdg = statp.tile([P, s], f32, tag="std")
            nc.scalar.activation(out=stdg[:], in_=vars_g[:], func=AF.Sqrt,
                                 bias=eps_t[:], scale=1.0 / H)
            rstd = statp.tile([P, s], f32, tag="rstd")
            nc.vector.reciprocal(out=rstd[:], in_=stdg[:])
            fc = fc_ps.tile([P, s * NUM_CLASSES], f32, space="PSUM")
            for st in states:
                st["rstd"] = rstd
                st["fc"] = fc
            prev_states = states
            prev_gst = {"s": s, "tb": tb, "fc": fc}

        # final group's pass B
        for st in prev_states:
            emit_b1(st)
            emit_b2(st)
        emit_outadd(prev_gst)

        out_view = out_d.rearrange("(t p) c -> p t c", p=P)
        acc_view = out_acc[:].rearrange("p (t c) -> p t c", c=NUM_CLASSES)
        nc.sync.dma_start(out_view, acc_view)

    nc.compile()
    return nc


def _ensure_ntff_hook():
    import sys, types
    try:
        from antenv.axon_hooks import get_axon_ntff_profile_hook  # noqa: F401
        return
    except ImportError:
        pass
    mod = types.ModuleType("antenv.axon_hooks")
    _hook = [None]
    mod.set_axon_ntff_profile_hook = lambda h: _hook.__setitem__(0, h)
    mod.get_axon_ntff_profile_hook = lambda: _hook[0]
    sys.modules["antenv.axon_hooks"] = mod
    try:
        import antenv
        antenv.axon_hooks = mod
    except ImportError:
        pass
    try:
        from trn_agent_boot.trn_boot import _ntff_profile_via_ctypes
        mod.set_axon_ntff_profile_hook(
            _ntff_profile_via_ctypes("/opt/axon/libaxon_pjrt.so"))
    except Exception:
        pass


# ----------------------------------------------------------------------------
# entry point
# ----------------------------------------------------------------------------
def kernel(x, edge_index, edge_weight, W1, b1, ln_g, ln_b, Wfc, bfc):
    global LAST_RESULTS
    from concourse.bass_utils import run_bass_kernel_spmd

    x16 = np.asarray(x, dtype=np.float32).astype(np.float16)
    meta = _preprocess(edge_index, edge_weight)
    IDXC = meta["IDXC"]

    if "prog" not in _PROGRAM_CACHE:
        _PROGRAM_CACHE["prog"] = _build_program()
    nc = _PROGRAM_CACHE["prog"]

    W1f = np.asarray(W1, np.float32)
    W1aug = np.zeros((IN_CH, HIDDEN + 1), dtype=np.float16)
    W1aug[:, :HIDDEN] = W1f.astype(np.float16)
    W1aug[:, HIDDEN] = (-W1f.mean(axis=1)).astype(np.float16)
    b1f = np.asarray(b1, np.float32).reshape(-1)
    c1 = (b1f - b1f.mean()).reshape(1, HIDDEN).astype(np.float32)

    rows = meta["node_tile"] * P + meta["node_slot"]
    xts = np.zeros((P, TILES * P), dtype=np.float16)
    xts[:, rows] = (x16.astype(np.float32)
                    * meta["selfnorm16"].astype(np.float32)[:, None]
                    ).astype(np.float16).T

    banks = {}
    for b in range(NBANK):
        blk = np.zeros((BANK, IN_CH), dtype=np.float16)
        seg = x16[b * BANK:(b + 1) * BANK]
        blk[:len(seg)] = seg
        banks[f"xb{b}"] = blk

    common = dict(
        banks,
        W1aug=W1aug,
        Wfc=np.asarray(Wfc, np.float32).astype(np.float16),
        c1=c1,
        ln_g=np.asarray(ln_g, np.float32).reshape(HIDDEN, 1),
        ln_b=np.asarray(ln_b, np.float32).reshape(HIDDEN, 1),
        bfc=np.asarray(bfc, np.float32).reshape(1, NUM_CLASSES),
        iota=np.arange(P, dtype=np.float16).reshape(1, P),
        idm=np.eye(P, dtype=np.float16),
    )
    MC = TPC * NBANK * K * 2
    in_maps = []
    for core in range(N_CORES):
        msl = slice(core * MC, (core + 1) * MC)
        ssl = slice(core * TPC * P, (core + 1) * TPC * P)
        in_maps.append(dict(
            common,
            idx=np.ascontiguousarray(meta["idx_all"][:, core * IDXC:(core + 1) * IDXC]),
            dstl=np.ascontiguousarray(meta["dstl_all"][:, msl]),
            normv=np.ascontiguousarray(meta["norm_all"][:, msl]),
            xts=np.ascontiguousarray(xts[:, ssl]),
        ))

    trace = bool(os.environ.get("KERNEL_TRACE"))
    if trace:
        _ensure_ntff_hook()
    res = run_bass_kernel_spmd(nc, in_maps, list(range(N_CORES)), trace=trace)
    LAST_RESULTS = res

    all_rows = np.concatenate([res.results[c]["out"] for c in range(N_CORES)],
                              axis=0)
    return np.ascontiguousarray(all_rows[rows].astype(np.float32))


# revision 37
# speedup vs baseline: 1.1461x; 1.1461x over previous
"""GCN classifier (GCNConv + LayerNorm + ReLU + Linear) on 8 Trainium2 NeuronCores.

v2 strategy (self-contained; sized for N=100000, E=1600000, 128 ch, 16 classes):
  out = LN((A @ x) @ W1 + b1).relu() @ Wfc + bfc,  A = normalized adjacency.

  Profiling insights driving this design (vs v1 baseline @ 2.48ms):
  - SWDGE descriptor generation on the Q7 cores is ~8ns/descriptor and was
    2.0ms serial on one core pair. Fix: num_swdge_queues=4, one gather call
    per source bank on its own queue_num -> 4 Q7 pairs generate in parallel.
  - DVE tensor_scalar/copy can enter 2-port perf mode which takes an
    exclusive lock on the SBUF port shared with GPSIMD -> one-hot builds
    were blocking descriptor generation (and vice versa), 4.2ms of DVE time.
    Fix: build one-hot slabs with tensor_tensor (never contends) using
    stride-0 repeat APs; PSUM evacuations / scaling moved to the ACT engine
    (own SBUF port).
  - fp32 matmuls are 4 cycles/row on the PE; fp16 is 1. Everything on the
    matmul path is fp16 (tolerance is 2e-2; fp16 keeps us ~1e-3).
  - Padding trimmed: nodes are packed into 888 tiles so every (tile, bank)
    cell fits exactly K=4 chunks of 128 edges (~7% pad vs ~30%); self-loops
    are not gathered at all - they stream as dense 128-row blocks from a
    tile-permuted fp16 copy of x via HWDGE (free of Q7 descriptor cost).
  - LayerNorm mean comes free as an extra (negated row-mean) column of the
    W1 matmul; LN affine + ReLU fold into one ACT op in transposed layout.
"""
import heapq
import os

import numpy as np

N_NODES = 100000
IN_CH = 128
HIDDEN = 128
NUM_CLASSES = 16
LN_EPS = 1e-5
N_CORES = 8
P = 128
BANK = 25000
NBANK = 4
K = 4                 # chunks per (tile, bank)
CPT = NBANK * K + 1   # chunks per tile (16 gather + 1 dense self block)
CELLCAP = K * P       # max edges per (tile, bank)
TILES = 848
TPC = TILES // N_CORES
GS = 8                # tiles per gather group

LAST_RESULTS = None
_PROGRAM_CACHE = {}


def _groups():
    out = []
    t = 0
    while t < TPC:
        s = min(GS, TPC - t)
        out.append((s, t))
        t += s
    return out


def _call_col_bases():
    """Column base (in 16-wide int16 idx columns) of each (group, bank) gather."""
    bases = []
    run = 0
    for s, _ in _groups():
        row = []
        for _b in range(NBANK):
            row.append(run)
            run += s * K * P // 16
        bases.append(row)
    return bases, run


# ----------------------------------------------------------------------------
# host-side preprocessing
# ----------------------------------------------------------------------------
def _assign_tiles(dst, eb, cnt_nb):
    """LPT-pack nodes into TILES tiles (<=128 nodes each), then repair so every
    (tile, bank) cell holds <= CELLCAP edges."""
    N = N_NODES
    cnt = cnt_nb.sum(axis=1)
    order = np.argsort(-cnt, kind="stable")
    heap = [(0, t) for t in range(TILES)]
    heapq.heapify(heap)
    node_cnt = np.zeros(TILES, dtype=np.int64)
    edge_sum = np.zeros(TILES, dtype=np.int64)
    node_tile = np.empty(N, dtype=np.int64)
    for nd in order:
        while True:
            s, t = heapq.heappop(heap)
            if node_cnt[t] < P:
                break
        node_tile[nd] = t
        node_cnt[t] += 1
        edge_sum[t] += cnt[nd]
        if node_cnt[t] < P:
            heapq.heappush(heap, (edge_sum[t], t))

    # repair per-bank overflows
    for _ in range(64):
        cell = np.zeros((TILES, NBANK), dtype=np.int64)
        np.add.at(cell, (node_tile[dst], eb), 1)
        over = np.argwhere(cell > CELLCAP)
        if len(over) == 0:
            break
        node_cnt = np.bincount(node_tile, minlength=TILES)
        for t, b in over:
            excess = cell[t, b] - CELLCAP
            if excess <= 0:
                continue
            nodes_t = np.where(node_tile == t)[0]
            cand = nodes_t[np.argsort(-cnt_nb[nodes_t, b], kind="stable")]
            for nd in cand:
                if excess <= 0:
                    break
                c_nd = cnt_nb[nd]
                if c_nd[b] == 0:
                    break
                ok = (node_cnt < P) & ((cell + c_nd[None, :]) <= CELLCAP).all(axis=1)
                ok[t] = False
                if not ok.any():
                    continue
                cand_t2 = np.where(ok)[0]
                t2 = cand_t2[np.argmin(cell[cand_t2].sum(axis=1))]
                node_tile[nd] = t2
                cell[t] -= c_nd
                cell[t2] += c_nd
                node_cnt[t] -= 1
                node_cnt[t2] += 1
                excess = cell[t, b] - CELLCAP
    else:
        raise RuntimeError("tile repair did not converge")

    # compact slots within each tile
    order2 = np.argsort(node_tile, kind="stable")
    tile_sorted = node_tile[order2]
    starts = np.zeros(TILES + 1, dtype=np.int64)
    np.cumsum(np.bincount(tile_sorted, minlength=TILES), out=starts[1:])
    node_slot = np.empty(N, dtype=np.int64)
    node_slot[order2] = np.arange(N) - starts[tile_sorted]
    assert (node_slot < P).all()
    return node_tile, node_slot


def _preprocess(edge_index, edge_weight):
    src = np.asarray(edge_index[0], dtype=np.int64)
    dst = np.asarray(edge_index[1], dtype=np.int64)
    w = np.asarray(edge_weight, dtype=np.float32)
    N = N_NODES

    deg = np.bincount(dst, weights=w.astype(np.float64), minlength=N) + 1.0
    dinv = (1.0 / np.sqrt(deg)).astype(np.float32)
    norm = (dinv[src] * w * dinv[dst]).astype(np.float32)
    selfnorm = (dinv.astype(np.float64) ** 2).astype(np.float32)  # 1/deg

    eb = src // BANK
    cnt_nb = np.zeros((N, NBANK), dtype=np.int64)
    np.add.at(cnt_nb, (dst, eb), 1)
    node_tile, node_slot = _assign_tiles(dst, eb, cnt_nb)

    # per-edge (tile, bank) cell position
    et = node_tile[dst]
    keys = et * NBANK + eb
    eorder = np.argsort(keys, kind="stable")
    keys_s = keys[eorder]
    cum = np.zeros(TILES * NBANK + 1, dtype=np.int64)
    np.cumsum(np.bincount(keys_s, minlength=TILES * NBANK), out=cum[1:])
    pos = np.arange(len(keys_s)) - cum[keys_s]
    kk = pos // P
    lane = pos % P
    assert (kk < K).all(), "cell overflow after repair"

    src_s = src[eorder]
    dst_s = dst[eorder]
    et_s = et[eorder]
    eb_s = eb[eorder]
    norm_s = norm[eorder]

    # tile-major metadata, duplicated-pair layout [128, tile*(16*2) + (bank*K+kk)*2 + {0,1}]
    # (pairs give every DVE operand an innermost stride-1 dim -> 2x perf mode)
    GCH = NBANK * K  # gathered chunks per tile (self handled via selfoh)
    MCOLS = TILES * GCH * 2
    mcol = (et_s * GCH + eb_s * K + kk) * 2
    dstl_all = np.zeros((P, MCOLS), dtype=np.float16)
    norm_all = np.zeros((P, MCOLS), dtype=np.float16)
    dstl_all[lane, mcol] = node_slot[dst_s].astype(np.float16)
    dstl_all[lane, mcol + 1] = dstl_all[lane, mcol]
    norm_all[lane, mcol] = norm_s.astype(np.float16)
    norm_all[lane, mcol + 1] = norm_all[lane, mcol]

    # self-loop contribution folded post-aggregation: transposed, pre-scaled
    # x rows in tile-permuted order; added into Ps [ch, d] by one DVE op/tile
    perm_rows = node_tile * P + node_slot
    selfnorm16 = selfnorm.astype(np.float16)

    # gather indices, call-major: per core, per (group, bank) call,
    # within call linear i = (j*K + kk)*128 + lane
    bases, IDXC = _call_col_bases()
    core = et_s // TPC
    tl = et_s % TPC
    g = tl // GS
    j = tl % GS
    cb = np.asarray([[bases[gi][bi] for bi in range(NBANK)]
                     for gi in range(len(bases))], dtype=np.int64)
    i_lin = (j * K + kk) * P + lane
    col16 = core * IDXC + cb[g, eb_s] + i_lin // 16
    row16 = i_lin % 16
    idx16 = np.zeros((16, N_CORES * IDXC), dtype=np.int16)
    idx16[row16, col16] = (src_s % BANK).astype(np.int16)
    idx_all = np.tile(idx16, (8, 1))

    return dict(
        idx_all=idx_all, norm_all=norm_all, dstl_all=dstl_all,
        perm_rows=perm_rows, selfnorm16=selfnorm16,
        node_tile=node_tile, node_slot=node_slot, IDXC=IDXC,
    )


# ----------------------------------------------------------------------------
# device program
# ----------------------------------------------------------------------------
def _build_program():
    from contextlib import ExitStack
    import concourse.bass as bass
    import concourse.tile as tile
    from concourse import bacc, mybir

    f32 = mybir.dt.float32
    f16 = mybir.dt.float16
    i16 = mybir.dt.int16
    H = HIDDEN
    GCH = NBANK * K
    MC = TPC * GCH * 2
    bases, IDXC = _call_col_bases()

    nc = bacc.Bacc("TRN2", target_bir_lowering=False, debug=False,
                   num_devices=N_CORES, num_swdge_queues=4)
    xb = [nc.dram_tensor(f"xb{b}", [BANK, IN_CH], f16, kind="ExternalInput").ap()
          for b in range(NBANK)]
    xts_d = nc.dram_tensor("xts", [P, TPC * P], f16, kind="ExternalInput").ap()
    idx_d = nc.dram_tensor("idx", [P, IDXC], i16, kind="ExternalInput").ap()
    dstl_d = nc.dram_tensor("dstl", [P, MC], f16, kind="ExternalInput").ap()
    norm_d = nc.dram_tensor("normv", [P, MC], f16, kind="ExternalInput").ap()
    w1_d = nc.dram_tensor("W1aug", [IN_CH, H + 1], f16, kind="ExternalInput").ap()
    wfc_d = nc.dram_tensor("Wfc", [H, NUM_CLASSES], f16, kind="ExternalInput").ap()
    c1_d = nc.dram_tensor("c1", [1, H], f32, kind="ExternalInput").ap()
    lng_d = nc.dram_tensor("ln_g", [H, 1], f32, kind="ExternalInput").ap()
    lnb_d = nc.dram_tensor("ln_b", [H, 1], f32, kind="ExternalInput").ap()
    bfc_d = nc.dram_tensor("bfc", [1, NUM_CLASSES], f32, kind="ExternalInput").ap()
    iota_d = nc.dram_tensor("iota", [1, P], f16, kind="ExternalInput").ap()
    idm_d = nc.dram_tensor("idm", [P, P], f16, kind="ExternalInput").ap()
    out_d = nc.dram_tensor("out", [TPC * P, NUM_CLASSES], f32,
                           kind="ExternalOutput").ap()

    def bcast(src_ap, parts=P):
        return bass.AP(tensor=src_ap.tensor, offset=src_ap.offset,
                       ap=[[0, parts]] + list(src_ap.ap[1:]))

    def rep_mid(ap2d, n):
        """[p, q] -> [p, n, q] with the middle dim broadcast (stride 0)."""
        a = list(ap2d.ap)
        return bass.AP(tensor=ap2d.tensor, offset=ap2d.offset,
                       ap=[a[0], [0, n], a[1]])

    # 4D APs for the one-hot slab build; every operand keeps an innermost
    # stride-1 dim of size 2 so the DVE can enter 2x_1P perf mode.
    def meta_rep4(ap2d):
        """paired meta [p, 2*GCH] -> [p, GCH, 64, 2]; value const along dim 64."""
        a = list(ap2d.ap)
        return bass.AP(tensor=ap2d.tensor, offset=ap2d.offset,
                       ap=[a[0], [2, GCH], [0, P // 2], [1, 2]])

    def iota_rep4(ap2d):
        """IOTA [p, 128] -> [p, GCH, 64, 2]; iota along the last two dims."""
        a = list(ap2d.ap)
        return bass.AP(tensor=ap2d.tensor, offset=ap2d.offset,
                       ap=[a[0], [0, GCH], [2, P // 2], [1, 2]])

    def slab4(ap2d):
        """slab [p, GCH*128] -> [p, GCH, 64, 2] contiguous."""
        a = list(ap2d.ap)
        return bass.AP(tensor=ap2d.tensor, offset=ap2d.offset,
                       ap=[a[0], [P, GCH], [2, P // 2], [1, 2]])

    AL = mybir.AluOpType
    AF = mybir.ActivationFunctionType

    with tile.TileContext(nc) as tc, ExitStack() as ctx:
        consts = ctx.enter_context(tc.tile_pool(name="consts", bufs=1))
        gpool = ctx.enter_context(tc.tile_pool(name="gather", bufs=3))
        ohp = ctx.enter_context(tc.tile_pool(name="onehot", bufs=3))
        sp = ctx.enter_context(tc.tile_pool(name="work", bufs=4))
        t1p = ctx.enter_context(tc.tile_pool(name="t1c", bufs=2 * GS))
        statp = ctx.enter_context(tc.tile_pool(name="stats", bufs=4 * GS))
        pp_ps = ctx.enter_context(tc.tile_pool(name="pp_ps", bufs=2, space="PSUM"))
        agg_ps = ctx.enter_context(tc.tile_pool(name="agg_ps", bufs=2, space="PSUM"))
        tr_ps = ctx.enter_context(tc.tile_pool(name="tr_ps", bufs=2, space="PSUM"))
        fc_ps = ctx.enter_context(tc.tile_pool(name="fc_ps", bufs=2, space="PSUM"))

        W1s = consts.tile([IN_CH, H + 1], f16)
        nc.sync.dma_start(W1s[:], w1_d[:])
        Wfcs = consts.tile([H, NUM_CLASSES], f16)
        nc.sync.dma_start(Wfcs[:], wfc_d[:])
        C1s = consts.tile([P, H], f32)
        nc.sync.dma_start(C1s[:], bcast(c1_d))
        Gcol = consts.tile([H, 1], f32)
        nc.sync.dma_start(Gcol[:], lng_d[:])
        Bcol = consts.tile([H, 1], f32)
        nc.sync.dma_start(Bcol[:], lnb_d[:])
        BFCs = consts.tile([P, NUM_CLASSES], f32)
        nc.sync.dma_start(BFCs[:], bcast(bfc_d))
        IOTA = consts.tile([P, P], f16)
        nc.sync.dma_start(IOTA[:], bcast(iota_d))
        idents = consts.tile([P, P], f16)
        nc.sync.dma_start(idents[:], idm_d[:])
        eps_t = consts.tile([P, 1], f32)
        nc.vector.memset(eps_t[:], LN_EPS)

        idx_s = consts.tile([P, IDXC], i16)
        nc.sync.dma_start(idx_s[:], idx_d[:])
        dstl_s = consts.tile([P, MC], f16)
        nc.sync.dma_start(dstl_s[:], dstl_d[:])
        norm_s = consts.tile([P, MC], f16)
        nc.sync.dma_start(norm_s[:], norm_d[:])

        out_acc = consts.tile([P, TPC * NUM_CLASSES], f32)

        # Software-pipelined emission: the PE stream is kept dense by skewing
        # every cross-engine round trip behind enough chunk-matmul work that
        # its dependencies are already resolved when the PE (or DVE/ACT FIFO
        # head) reaches it.
        def emit_add(st):
            # fold self-loop contribution: Ps2 = Ps + (selfnorm-scaled x^T)
            Ps2 = sp.tile([IN_CH, P], f16, tag="Ps2")
            j = st["j"]
            nc.vector.tensor_tensor(
                out=Ps2[:], in0=st["Ps"][:],
                in1=st["xtg"][:, j * P:(j + 1) * P], op=AL.add)
            st["Ps2"] = Ps2

        def emit_agg(st):
            agg = agg_ps.tile([P, H + 1], f32, space="PSUM")
            nc.tensor.matmul(agg[:], lhsT=st["Ps2"][:], rhs=W1s[:],
                             start=True, stop=True)
            st["agg"] = agg

        def emit_stats(st):
            agg = st["agg"]
            mu = statp.tile([P, 1], f32, tag="mu")
            nc.scalar.activation(out=mu[:], in_=agg[:, H:H + 1], func=AF.Copy)
            t1c = t1p.tile([P, H], f32, tag="t1c")
            nc.vector.scalar_tensor_tensor(
                out=t1c[:], in0=agg[:, 0:H], scalar=mu[:], in1=C1s[:],
                op0=AL.add, op1=AL.add)
            sq = sp.tile([P, H], f32, tag="sq")
            nc.scalar.activation(out=sq[:], in_=t1c[:], func=AF.Square,
                                 accum_out=st["vars"][:, st["j"]:st["j"] + 1])
            st["t1c"] = t1c

        def emit_b1(st):
            t1n = sp.tile([P, H], f16, tag="t1n")
            nc.scalar.activation(out=t1n[:], in_=st["t1c"][:], func=AF.Copy,
                                 scale=st["rstd"][:, st["j"]:st["j"] + 1])
            yT = tr_ps.tile([H, P], f16, space="PSUM")
            nc.tensor.transpose(out=yT[:], in_=t1n[:], identity=idents[:])
            st["yT"] = yT

        def emit_b2(st):
            hrT = sp.tile([H, P], f16, tag="hrT")
            nc.scalar.activation(out=hrT[:], in_=st["yT"][:], func=AF.Relu,
                                 scale=Gcol[:], bias=Bcol[:])
            j = st["j"]
            nc.tensor.matmul(st["fc"][:, j * NUM_CLASSES:(j + 1) * NUM_CLASSES],
                             lhsT=hrT[:], rhs=Wfcs[:], start=True, stop=True)

        def emit_outadd(gst):
            s, tb, fc = gst["s"], gst["tb"], gst["fc"]
            oslice = out_acc[:, tb * NUM_CLASSES:(tb + s) * NUM_CLASSES]
            o3 = oslice.rearrange("p (t c) -> p t c", c=NUM_CLASSES)
            f3 = fc[:, 0:s * NUM_CLASSES].rearrange("p (t c) -> p t c",
                                                    c=NUM_CLASSES)
            nc.vector.tensor_tensor(out=o3, in0=f3,
                                    in1=rep_mid(BFCs[:], s), op=AL.add)

        prev_states = None   # tile states of the previous group (pass B pending)
        prev_gst = None
        for gi, (s, tb) in enumerate(_groups()):
            Gg = gpool.tile([P, s * GCH, IN_CH], f16, tag="Gg")
            for b in range(NBANK):
                n = s * K * P
                cbase = bases[gi][b]
                nc.gpsimd.dma_gather(
                    out_ap=Gg[:, b * s * K:(b + 1) * s * K, :],
                    in_ap=xb[b][:],
                    idxs_ap=idx_s[:, cbase:cbase + n // 16],
                    num_idxs=n, num_idxs_reg=n, elem_size=IN_CH,
                    single_packet=False, queue_num=b,
                )
            # self-loop columns (pre-scaled x^T, tile-permuted) via HWDGE
            xtg = gpool.tile([P, s * P], f16, tag="xtg")
            xt_in = bass.AP(tensor=xts_d.tensor,
                            offset=xts_d.offset + tb * P,
                            ap=[[TPC * P, P], [1, s * P]])
            nc.sync.dma_start(xtg[:], xt_in)

            vars_g = statp.tile([P, s], f32, tag="vars")
            states = []
            for j in range(s):
                t = tb + j
                mc0 = t * GCH * 2
                dsl = dstl_s[:, mc0:mc0 + GCH * 2]
                nsl = norm_s[:, mc0:mc0 + GCH * 2]
                tmp = ohp.tile([P, GCH * P], f16, tag="tmp")
                nc.vector.tensor_tensor(out=slab4(tmp[:]), in0=meta_rep4(dsl),
                                        in1=iota_rep4(IOTA[:]), op=AL.is_equal)
                ohs = ohp.tile([P, GCH * P], f16, tag="ohs")
                nc.vector.tensor_tensor(out=slab4(ohs[:]), in0=slab4(tmp[:]),
                                        in1=meta_rep4(nsl), op=AL.mult)

                Pp = pp_ps.tile([IN_CH, P], f32, space="PSUM")
                for c in range(GCH):
                    gcol = (c // K) * s * K + j * K + (c % K)
                    nc.tensor.matmul(Pp[:], lhsT=Gg[:, gcol, :],
                                     rhs=ohs[:, c * P:(c + 1) * P],
                                     start=(c == 0), stop=(c == GCH - 1))
                Ps = sp.tile([IN_CH, P], f16, tag="Ps")
                nc.scalar.activation(out=Ps[:], in_=Pp[:], func=AF.Copy)
                states.append({"j": j, "Ps": Ps, "vars": vars_g, "xtg": xtg})

                # skewed tails: add+agg one tile behind, stats two tiles behind
                if j >= 1:
                    emit_add(states[j - 1])
                    emit_agg(states[j - 1])
                if j >= 2:
                    emit_stats(states[j - 2])
                # previous group's pass B: b1 at tile j, b2 one tile later
                if prev_states is not None:
                    if j < len(prev_states):
                        emit_b1(prev_states[j])
                    if 1 <= j <= len(prev_states):
                        emit_b2(prev_states[j - 1])

            # group-boundary flush of pass A
            emit_add(states[s - 1])
            emit_agg(states[s - 1])
            for jj in range(max(0, s - 2), s):
                emit_stats(states[jj])
            if prev_states is not None:
                for k in range(s, len(prev_states)):
                    emit_b1(prev_states[k])
                for k in range(max(1, s), len(prev_states) + 1):
                    emit_b2(prev_states[k - 1])
                emit_outadd(prev_gst)

            stdg = statp.tile([P, s], f32, tag="std")
            nc.scalar.activation(out=stdg[:], in_=vars_g[:], func=AF.Sqrt,
                                 bias=eps_t[:], scale=1.0 / H)
            rstd = statp.tile([P, s], f32, tag="rstd")
            nc.vector.reciprocal(out=rstd[:], in_=stdg[:])
            fc = fc_ps.tile([P, s * NUM_CLASSES], f32, space="PSUM")
            for st in states:
                st["rstd"] = rstd
                st["fc"] = fc
            prev_states = states
            prev_gst = {"s": s, "tb": tb, "fc": fc}

        # final group's pass B
        for st in prev_states:
            emit_b1(st)
            emit_b2(st)
        emit_outadd(prev_gst)

        out_view = out_d.rearrange("(t p) c -> p t c", p=P)
        acc_view = out_acc[:].rearrange("p (t c) -> p t c", c=NUM_CLASSES)
        nc.sync.dma_start(out_view, acc_view)

    nc.compile()
    return nc


def _ensure_ntff_hook():
    import sys, types
    try:
        from antenv.axon_hooks import get_axon_ntff_profile_hook  # noqa: F401
        return
    except ImportError:
        pass
    mod = types.ModuleType("antenv.axon_hooks")
    _hook = [None]
    mod.set_axon_ntff_profile_hook = lambda h: _hook.__setitem__(0, h)
    mod.get_axon_ntff_profile_hook = lambda: _hook[0]
    sys.modules["antenv.axon_hooks"] = mod
    try:
        import antenv
        antenv.axon_hooks = mod
    except ImportError:
        pass
    try:
        from trn_agent_boot.trn_boot import _ntff_profile_via_ctypes
        mod.set_axon_ntff_profile_hook(
            _ntff_profile_via_ctypes("/opt/axon/libaxon_pjrt.so"))
    except Exception:
        pass


# ----------------------------------------------------------------------------
# entry point
# ----------------------------------------------------------------------------
def kernel(x, edge_index, edge_weight, W1, b1, ln_g, ln_b, Wfc, bfc):
    global LAST_RESULTS
    from concourse.bass_utils import run_bass_kernel_spmd

    x16 = np.asarray(x, dtype=np.float32).astype(np.float16)
    meta = _preprocess(edge_index, edge_weight)
    IDXC = meta["IDXC"]

    if "prog" not in _PROGRAM_CACHE:
        _PROGRAM_CACHE["prog"] = _build_program()
    nc = _PROGRAM_CACHE["prog"]

    W1f = np.asarray(W1, np.float32)
    W1aug = np.zeros((IN_CH, HIDDEN + 1), dtype=np.float16)
    W1aug[:, :HIDDEN] = W1f.astype(np.float16)
    W1aug[:, HIDDEN] = (-W1f.mean(axis=1)).astype(np.float16)
    b1f = np.asarray(b1, np.float32).reshape(-1)
    c1 = (b1f - b1f.mean()).reshape(1, HIDDEN).astype(np.float32)

    rows = meta["node_tile"] * P + meta["node_slot"]
    xts = np.zeros((P, TILES * P), dtype=np.float16)
    xts[:, rows] = (x16.astype(np.float32)
                    * meta["selfnorm16"].astype(np.float32)[:, None]
                    ).astype(np.float16).T

    banks = {}
    for b in range(NBANK):
        blk = np.zeros((BANK, IN_CH), dtype=np.float16)
        seg = x16[b * BANK:(b + 1) * BANK]
        blk[:len(seg)] = seg
        banks[f"xb{b}"] = blk

    common = dict(
        banks,
        W1aug=W1aug,
        Wfc=np.asarray(Wfc, np.float32).astype(np.float16),
        c1=c1,
        ln_g=np.asarray(ln_g, np.float32).reshape(HIDDEN, 1),
        ln_b=np.asarray(ln_b, np.float32).reshape(HIDDEN, 1),
        bfc=np.asarray(bfc, np.float32).reshape(1, NUM_CLASSES),
        iota=np.arange(P, dtype=np.float16).reshape(1, P),
        idm=np.eye(P, dtype=np.float16),
    )
    MC = TPC * NBANK * K * 2
    in_maps = []
    for core in range(N_CORES):
        msl = slice(core * MC, (core + 1) * MC)
        ssl = slice(core * TPC * P, (core + 1) * TPC * P)
        in_maps.append(dict(
            common,
            idx=np.ascontiguousarray(meta["idx_all"][:, core * IDXC:(core + 1) * IDXC]),
            dstl=np.ascontiguousarray(meta["dstl_all"][:, msl]),
            normv=np.ascontiguousarray(meta["norm_all"][:, msl]),
            xts=np.ascontiguousarray(xts[:, ssl]),
        ))

    trace = bool(os.environ.get("KERNEL_TRACE"))
    if trace:
        _ensure_ntff_hook()
    res = run_bass_kernel_spmd(nc, in_maps, list(range(N_CORES)), trace=trace)
    LAST_RESULTS = res

    all_rows = np.concatenate([res.results[c]["out"] for c in range(N_CORES)],
                              axis=0)
    return np.ascontiguousarray(all_rows[rows].astype(np.float32))


# revision 39
# speedup vs baseline: 1.1842x; 1.0333x over previous
"""GCN classifier (GCNConv + LayerNorm + ReLU + Linear) on 8 Trainium2 NeuronCores.

v2 strategy (self-contained; sized for N=100000, E=1600000, 128 ch, 16 classes):
  out = LN((A @ x) @ W1 + b1).relu() @ Wfc + bfc,  A = normalized adjacency.

  Profiling insights driving this design (vs v1 baseline @ 2.48ms):
  - SWDGE descriptor generation on the Q7 cores is ~8ns/descriptor and was
    2.0ms serial on one core pair. Fix: num_swdge_queues=4, one gather call
    per source bank on its own queue_num -> 4 Q7 pairs generate in parallel.
  - DVE tensor_scalar/copy can enter 2-port perf mode which takes an
    exclusive lock on the SBUF port shared with GPSIMD -> one-hot builds
    were blocking descriptor generation (and vice versa), 4.2ms of DVE time.
    Fix: build one-hot slabs with tensor_tensor (never contends) using
    stride-0 repeat APs; PSUM evacuations / scaling moved to the ACT engine
    (own SBUF port).
  - fp32 matmuls are 4 cycles/row on the PE; fp16 is 1. Everything on the
    matmul path is fp16 (tolerance is 2e-2; fp16 keeps us ~1e-3).
  - Padding trimmed: nodes are packed into 888 tiles so every (tile, bank)
    cell fits exactly K=4 chunks of 128 edges (~7% pad vs ~30%); self-loops
    are not gathered at all - they stream as dense 128-row blocks from a
    tile-permuted fp16 copy of x via HWDGE (free of Q7 descriptor cost).
  - LayerNorm mean comes free as an extra (negated row-mean) column of the
    W1 matmul; LN affine + ReLU fold into one ACT op in transposed layout.
"""
import heapq
import os

import numpy as np

N_NODES = 100000
IN_CH = 128
HIDDEN = 128
NUM_CLASSES = 16
LN_EPS = 1e-5
N_CORES = 8
P = 128
BANK = 25000
NBANK = 4
K = 4                 # chunks per (tile, bank)
CPT = NBANK * K + 1   # chunks per tile (16 gather + 1 dense self block)
CELLCAP = K * P       # max edges per (tile, bank)
TILES = 848
TPC = TILES // N_CORES
GS = 8                # tiles per gather group

LAST_RESULTS = None
_PROGRAM_CACHE = {}


def _groups():
    out = []
    t = 0
    while t < TPC:
        s = min(GS, TPC - t)
        out.append((s, t))
        t += s
    return out


def _call_col_bases():
    """Column base (in 16-wide int16 idx columns) of each (group, bank) gather."""
    bases = []
    run = 0
    for s, _ in _groups():
        row = []
        for _b in range(NBANK):
            row.append(run)
            run += s * K * P // 16
        bases.append(row)
    return bases, run


# ----------------------------------------------------------------------------
# host-side preprocessing
# ----------------------------------------------------------------------------
def _assign_tiles(dst, eb, cnt_nb):
    """LPT-pack nodes into TILES tiles (<=128 nodes each), then repair so every
    (tile, bank) cell holds <= CELLCAP edges."""
    N = N_NODES
    cnt = cnt_nb.sum(axis=1)
    order = np.argsort(-cnt, kind="stable")
    heap = [(0, t) for t in range(TILES)]
    heapq.heapify(heap)
    node_cnt = np.zeros(TILES, dtype=np.int64)
    edge_sum = np.zeros(TILES, dtype=np.int64)
    node_tile = np.empty(N, dtype=np.int64)
    for nd in order:
        while True:
            s, t = heapq.heappop(heap)
            if node_cnt[t] < P:
                break
        node_tile[nd] = t
        node_cnt[t] += 1
        edge_sum[t] += cnt[nd]
        if node_cnt[t] < P:
            heapq.heappush(heap, (edge_sum[t], t))

    # repair per-bank overflows
    for _ in range(64):
        cell = np.zeros((TILES, NBANK), dtype=np.int64)
        np.add.at(cell, (node_tile[dst], eb), 1)
        over = np.argwhere(cell > CELLCAP)
        if len(over) == 0:
            break
        node_cnt = np.bincount(node_tile, minlength=TILES)
        for t, b in over:
            excess = cell[t, b] - CELLCAP
            if excess <= 0:
                continue
            nodes_t = np.where(node_tile == t)[0]
            cand = nodes_t[np.argsort(-cnt_nb[nodes_t, b], kind="stable")]
            for nd in cand:
                if excess <= 0:
                    break
                c_nd = cnt_nb[nd]
                if c_nd[b] == 0:
                    break
                ok = (node_cnt < P) & ((cell + c_nd[None, :]) <= CELLCAP).all(axis=1)
                ok[t] = False
                if not ok.any():
                    continue
                cand_t2 = np.where(ok)[0]
                t2 = cand_t2[np.argmin(cell[cand_t2].sum(axis=1))]
                node_tile[nd] = t2
                cell[t] -= c_nd
                cell[t2] += c_nd
                node_cnt[t] -= 1
                node_cnt[t2] += 1
                excess = cell[t, b] - CELLCAP
    else:
        raise RuntimeError("tile repair did not converge")

    # compact slots within each tile
    order2 = np.argsort(node_tile, kind="stable")
    tile_sorted = node_tile[order2]
    starts = np.zeros(TILES + 1, dtype=np.int64)
    np.cumsum(np.bincount(tile_sorted, minlength=TILES), out=starts[1:])
    node_slot = np.empty(N, dtype=np.int64)
    node_slot[order2] = np.arange(N) - starts[tile_sorted]
    assert (node_slot < P).all()
    return node_tile, node_slot


def _preprocess(edge_index, edge_weight):
    src = np.asarray(edge_index[0], dtype=np.int64)
    dst = np.asarray(edge_index[1], dtype=np.int64)
    w = np.asarray(edge_weight, dtype=np.float32)
    N = N_NODES

    deg = np.bincount(dst, weights=w.astype(np.float64), minlength=N) + 1.0
    dinv = (1.0 / np.sqrt(deg)).astype(np.float32)
    norm = (dinv[src] * w * dinv[dst]).astype(np.float32)
    selfnorm = (dinv.astype(np.float64) ** 2).astype(np.float32)  # 1/deg

    eb = src // BANK
    cnt_nb = np.zeros((N, NBANK), dtype=np.int64)
    np.add.at(cnt_nb, (dst, eb), 1)
    node_tile, node_slot = _assign_tiles(dst, eb, cnt_nb)

    # per-edge (tile, bank) cell position
    et = node_tile[dst]
    keys = et * NBANK + eb
    eorder = np.argsort(keys, kind="stable")
    keys_s = keys[eorder]
    cum = np.zeros(TILES * NBANK + 1, dtype=np.int64)
    np.cumsum(np.bincount(keys_s, minlength=TILES * NBANK), out=cum[1:])
    pos = np.arange(len(keys_s)) - cum[keys_s]
    kk = pos // P
    lane = pos % P
    assert (kk < K).all(), "cell overflow after repair"

    src_s = src[eorder]
    dst_s = dst[eorder]
    et_s = et[eorder]
    eb_s = eb[eorder]
    norm_s = norm[eorder]

    # tile-major metadata, duplicated-pair layout [128, tile*(16*2) + (bank*K+kk)*2 + {0,1}]
    # (pairs give every DVE operand an innermost stride-1 dim -> 2x perf mode)
    GCH = NBANK * K  # gathered chunks per tile (self handled via selfoh)
    MCOLS = TILES * GCH * 2
    mcol = (et_s * GCH + eb_s * K + kk) * 2
    dstl_all = np.zeros((P, MCOLS), dtype=np.float16)
    norm_all = np.zeros((P, MCOLS), dtype=np.float16)
    dstl_all[lane, mcol] = node_slot[dst_s].astype(np.float16)
    dstl_all[lane, mcol + 1] = dstl_all[lane, mcol]
    norm_all[lane, mcol] = norm_s.astype(np.float16)
    norm_all[lane, mcol + 1] = norm_all[lane, mcol]

    # self-loop contribution folded post-aggregation: transposed, pre-scaled
    # x rows in tile-permuted order; added into Ps [ch, d] by one DVE op/tile
    perm_rows = node_tile * P + node_slot
    selfnorm16 = selfnorm.astype(np.float16)

    # gather indices, call-major: per core, per (group, bank) call,
    # within call linear i = (j*K + kk)*128 + lane
    bases, IDXC = _call_col_bases()
    core = et_s // TPC
    tl = et_s % TPC
    g = tl // GS
    j = tl % GS
    cb = np.asarray([[bases[gi][bi] for bi in range(NBANK)]
                     for gi in range(len(bases))], dtype=np.int64)
    i_lin = (j * K + kk) * P + lane
    col16 = core * IDXC + cb[g, eb_s] + i_lin // 16
    row16 = i_lin % 16
    idx16 = np.zeros((16, N_CORES * IDXC), dtype=np.int16)
    idx16[row16, col16] = (src_s % BANK).astype(np.int16)
    idx_all = np.tile(idx16, (8, 1))

    return dict(
        idx_all=idx_all, norm_all=norm_all, dstl_all=dstl_all,
        perm_rows=perm_rows, selfnorm16=selfnorm16,
        node_tile=node_tile, node_slot=node_slot, IDXC=IDXC,
    )


# ----------------------------------------------------------------------------
# device program
# ----------------------------------------------------------------------------
def _build_program():
    from contextlib import ExitStack
    import concourse.bass as bass
    import concourse.tile as tile
    from concourse import bacc, mybir

    f32 = mybir.dt.float32
    f16 = mybir.dt.float16
    i16 = mybir.dt.int16
    H = HIDDEN
    GCH = NBANK * K
    MC = TPC * GCH * 2
    bases, IDXC = _call_col_bases()

    nc = bacc.Bacc("TRN2", target_bir_lowering=False, debug=False,
                   num_devices=N_CORES, num_swdge_queues=4)
    xb = [nc.dram_tensor(f"xb{b}", [BANK, IN_CH], f16, kind="ExternalInput").ap()
          for b in range(NBANK)]
    xts_d = nc.dram_tensor("xts", [P, TPC * P], f16, kind="ExternalInput").ap()
    idx_d = nc.dram_tensor("idx", [P, IDXC], i16, kind="ExternalInput").ap()
    dstl_d = nc.dram_tensor("dstl", [P, MC], f16, kind="ExternalInput").ap()
    norm_d = nc.dram_tensor("normv", [P, MC], f16, kind="ExternalInput").ap()
    w1_d = nc.dram_tensor("W1aug", [IN_CH, H + 1], f16, kind="ExternalInput").ap()
    wfc_d = nc.dram_tensor("Wfc", [H, NUM_CLASSES], f16, kind="ExternalInput").ap()
    c1_d = nc.dram_tensor("c1", [1, H], f32, kind="ExternalInput").ap()
    lng_d = nc.dram_tensor("ln_g", [H, 1], f32, kind="ExternalInput").ap()
    lnb_d = nc.dram_tensor("ln_b", [H, 1], f32, kind="ExternalInput").ap()
    bfc_d = nc.dram_tensor("bfc", [1, NUM_CLASSES], f32, kind="ExternalInput").ap()
    iota_d = nc.dram_tensor("iota", [1, P], f16, kind="ExternalInput").ap()
    idm_d = nc.dram_tensor("idm", [P, P], f16, kind="ExternalInput").ap()
    out_d = nc.dram_tensor("out", [TPC * P, NUM_CLASSES], f32,
                           kind="ExternalOutput").ap()

    def bcast(src_ap, parts=P):
        return bass.AP(tensor=src_ap.tensor, offset=src_ap.offset,
                       ap=[[0, parts]] + list(src_ap.ap[1:]))

    def rep_mid(ap2d, n):
        """[p, q] -> [p, n, q] with the middle dim broadcast (stride 0)."""
        a = list(ap2d.ap)
        return bass.AP(tensor=ap2d.tensor, offset=ap2d.offset,
                       ap=[a[0], [0, n], a[1]])

    # 4D APs for the one-hot slab build; every operand keeps an innermost
    # stride-1 dim of size 2 so the DVE can enter 2x_1P perf mode.
    def meta_rep4(ap2d):
        """paired meta [p, 2*GCH] -> [p, GCH, 64, 2]; value const along dim 64."""
        a = list(ap2d.ap)
        return bass.AP(tensor=ap2d.tensor, offset=ap2d.offset,
                       ap=[a[0], [2, GCH], [0, P // 2], [1, 2]])

    def iota_rep4(ap2d):
        """IOTA [p, 128] -> [p, GCH, 64, 2]; iota along the last two dims."""
        a = list(ap2d.ap)
        return bass.AP(tensor=ap2d.tensor, offset=ap2d.offset,
                       ap=[a[0], [0, GCH], [2, P // 2], [1, 2]])

    def slab4(ap2d):
        """slab [p, GCH*128] -> [p, GCH, 64, 2] contiguous."""
        a = list(ap2d.ap)
        return bass.AP(tensor=ap2d.tensor, offset=ap2d.offset,
                       ap=[a[0], [P, GCH], [2, P // 2], [1, 2]])

    AL = mybir.AluOpType
    AF = mybir.ActivationFunctionType

    with tile.TileContext(nc) as tc, ExitStack() as ctx:
        consts = ctx.enter_context(tc.tile_pool(name="consts", bufs=1))
        gpool = ctx.enter_context(tc.tile_pool(name="gather", bufs=3))
        ohp = ctx.enter_context(tc.tile_pool(name="onehot", bufs=3))
        sp = ctx.enter_context(tc.tile_pool(name="work", bufs=4))
        t1p = ctx.enter_context(tc.tile_pool(name="t1c", bufs=2 * GS))
        statp = ctx.enter_context(tc.tile_pool(name="stats", bufs=4 * GS))
        pp_ps = ctx.enter_context(tc.tile_pool(name="pp_ps", bufs=2, space="PSUM"))
        agg_ps = ctx.enter_context(tc.tile_pool(name="agg_ps", bufs=2, space="PSUM"))
        tr_ps = ctx.enter_context(tc.tile_pool(name="tr_ps", bufs=2, space="PSUM"))
        fc_ps = ctx.enter_context(tc.tile_pool(name="fc_ps", bufs=2, space="PSUM"))

        W1s = consts.tile([IN_CH, H + 1], f16)
        nc.sync.dma_start(W1s[:], w1_d[:])
        Wfcs = consts.tile([H, NUM_CLASSES], f16)
        nc.sync.dma_start(Wfcs[:], wfc_d[:])
        C1s = consts.tile([P, H], f32)
        nc.sync.dma_start(C1s[:], bcast(c1_d))
        Gcol = consts.tile([H, 1], f32)
        nc.sync.dma_start(Gcol[:], lng_d[:])
        Bcol = consts.tile([H, 1], f32)
        nc.sync.dma_start(Bcol[:], lnb_d[:])
        BFCs = consts.tile([P, NUM_CLASSES], f32)
        nc.sync.dma_start(BFCs[:], bcast(bfc_d))
        IOTA = consts.tile([P, P], f16)
        nc.sync.dma_start(IOTA[:], bcast(iota_d))
        idents = consts.tile([P, P], f16)
        nc.sync.dma_start(idents[:], idm_d[:])
        eps_t = consts.tile([P, 1], f32)
        nc.vector.memset(eps_t[:], LN_EPS)

        # metadata tiles loaded in per-group slices inside the loop so group 0
        # starts as soon as its own slice lands
        idx_s = consts.tile([P, IDXC], i16)
        dstl_s = consts.tile([P, MC], f16)
        norm_s = consts.tile([P, MC], f16)

        out_acc = consts.tile([P, TPC * NUM_CLASSES], f32)

        # Software-pipelined emission: the PE stream is kept dense by skewing
        # every cross-engine round trip behind enough chunk-matmul work that
        # its dependencies are already resolved when the PE (or DVE/ACT FIFO
        # head) reaches it.
        def emit_add(st):
            # fold self-loop contribution: Ps2 = Ps + (selfnorm-scaled x^T)
            Ps2 = sp.tile([IN_CH, P], f16, tag="Ps2")
            j = st["j"]
            nc.vector.tensor_tensor(
                out=Ps2[:], in0=st["Ps"][:],
                in1=st["xtg"][:, j * P:(j + 1) * P], op=AL.add)
            st["Ps2"] = Ps2

        def emit_agg(st):
            agg = agg_ps.tile([P, H + 1], f32, space="PSUM")
            nc.tensor.matmul(agg[:], lhsT=st["Ps2"][:], rhs=W1s[:],
                             start=True, stop=True)
            st["agg"] = agg

        def emit_stats(st):
            agg = st["agg"]
            mu = statp.tile([P, 1], f32, tag="mu")
            nc.scalar.activation(out=mu[:], in_=agg[:, H:H + 1], func=AF.Copy)
            t1c = t1p.tile([P, H], f32, tag="t1c")
            nc.vector.scalar_tensor_tensor(
                out=t1c[:], in0=agg[:, 0:H], scalar=mu[:], in1=C1s[:],
                op0=AL.add, op1=AL.add)
            sq = sp.tile([P, H], f32, tag="sq")
            nc.scalar.activation(out=sq[:], in_=t1c[:], func=AF.Square,
                                 accum_out=st["vars"][:, st["j"]:st["j"] + 1])
            st["t1c"] = t1c

        def emit_b1(st):
            t1n = sp.tile([P, H], f16, tag="t1n")
            nc.scalar.activation(out=t1n[:], in_=st["t1c"][:], func=AF.Copy,
                                 scale=st["rstd"][:, st["j"]:st["j"] + 1])
            yT = tr_ps.tile([H, P], f16, space="PSUM")
            nc.tensor.transpose(out=yT[:], in_=t1n[:], identity=idents[:])
            st["yT"] = yT

        def emit_b2(st):
            hrT = sp.tile([H, P], f16, tag="hrT")
            nc.scalar.activation(out=hrT[:], in_=st["yT"][:], func=AF.Relu,
                                 scale=Gcol[:], bias=Bcol[:])
            j = st["j"]
            nc.tensor.matmul(st["fc"][:, j * NUM_CLASSES:(j + 1) * NUM_CLASSES],
                             lhsT=hrT[:], rhs=Wfcs[:], start=True, stop=True)

        def emit_outadd(gst):
            s, tb, fc = gst["s"], gst["tb"], gst["fc"]
            oslice = out_acc[:, tb * NUM_CLASSES:(tb + s) * NUM_CLASSES]
            o3 = oslice.rearrange("p (t c) -> p t c", c=NUM_CLASSES)
            f3 = fc[:, 0:s * NUM_CLASSES].rearrange("p (t c) -> p t c",
                                                    c=NUM_CLASSES)
            nc.vector.tensor_tensor(out=o3, in0=f3,
                                    in1=rep_mid(BFCs[:], s), op=AL.add)

        prev_states = None   # tile states of the previous group (pass B pending)
        prev_gst = None
        for gi, (s, tb) in enumerate(_groups()):
            ic0 = bases[gi][0]
            ic1 = bases[gi][NBANK - 1] + s * K * P // 16
            nc.sync.dma_start(idx_s[:, ic0:ic1], idx_d[:, ic0:ic1])
            mc_lo = tb * GCH * 2
            mc_hi = (tb + s) * GCH * 2
            nc.sync.dma_start(dstl_s[:, mc_lo:mc_hi], dstl_d[:, mc_lo:mc_hi])
            nc.sync.dma_start(norm_s[:, mc_lo:mc_hi], norm_d[:, mc_lo:mc_hi])

            Gg = gpool.tile([P, s * GCH, IN_CH], f16, tag="Gg")
            for b in range(NBANK):
                n = s * K * P
                cbase = bases[gi][b]
                nc.gpsimd.dma_gather(
                    out_ap=Gg[:, b * s * K:(b + 1) * s * K, :],
                    in_ap=xb[b][:],
                    idxs_ap=idx_s[:, cbase:cbase + n // 16],
                    num_idxs=n, num_idxs_reg=n, elem_size=IN_CH,
                    single_packet=False, queue_num=b,
                )
            # self-loop columns (pre-scaled x^T, tile-permuted) via HWDGE
            xtg = gpool.tile([P, s * P], f16, tag="xtg")
            xt_in = bass.AP(tensor=xts_d.tensor,
                            offset=xts_d.offset + tb * P,
                            ap=[[TPC * P, P], [1, s * P]])
            nc.sync.dma_start(xtg[:], xt_in)

            vars_g = statp.tile([P, s], f32, tag="vars")
            states = []
            for j in range(s):
                t = tb + j
                mc0 = t * GCH * 2
                dsl = dstl_s[:, mc0:mc0 + GCH * 2]
                nsl = norm_s[:, mc0:mc0 + GCH * 2]
                tmp = ohp.tile([P, GCH * P], f16, tag="tmp")
                nc.vector.tensor_tensor(out=slab4(tmp[:]), in0=meta_rep4(dsl),
                                        in1=iota_rep4(IOTA[:]), op=AL.is_equal)
                ohs = ohp.tile([P, GCH * P], f16, tag="ohs")
                nc.vector.tensor_tensor(out=slab4(ohs[:]), in0=slab4(tmp[:]),
                                        in1=meta_rep4(nsl), op=AL.mult)

                Pp = pp_ps.tile([IN_CH, P], f32, space="PSUM")
                for c in range(GCH):
                    gcol = (c // K) * s * K + j * K + (c % K)
                    nc.tensor.matmul(Pp[:], lhsT=Gg[:, gcol, :],
                                     rhs=ohs[:, c * P:(c + 1) * P],
                                     start=(c == 0), stop=(c == GCH - 1))
                Ps = sp.tile([IN_CH, P], f16, tag="Ps")
                nc.scalar.activation(out=Ps[:], in_=Pp[:], func=AF.Copy)
                states.append({"j": j, "Ps": Ps, "vars": vars_g, "xtg": xtg})

                # skewed tails: add+agg one tile behind, stats two tiles behind
                if j >= 1:
                    emit_add(states[j - 1])
                    emit_agg(states[j - 1])
                if j >= 2:
                    emit_stats(states[j - 2])
                # previous group's pass B: b1 at tile j, b2 one tile later
                if prev_states is not None:
                    if j < len(prev_states):
                        emit_b1(prev_states[j])
                    if 1 <= j <= len(prev_states):
                        emit_b2(prev_states[j - 1])

            # group-boundary flush of pass A
            emit_add(states[s - 1])
            emit_agg(states[s - 1])
            for jj in range(max(0, s - 2), s):
                emit_stats(states[jj])
            if prev_states is not None:
                for k in range(s, len(prev_states)):
                    emit_b1(prev_states[k])
                for k in range(max(1, s), len(prev_states) + 1):
                    emit_b2(prev_states[k - 1])
                emit_outadd(prev_gst)

            stdg = statp.tile([P, s], f32, tag="std")
            nc.scalar.activation(out=stdg[:], in_=vars_g[:], func=AF.Sqrt,
                                 bias=eps_t[:], scale=1.0 / H)
            rstd = statp.tile([P, s], f32, tag="rstd")
            nc.vector.reciprocal(out=rstd[:], in_=stdg[:])
            fc = fc_ps.tile([P, s * NUM_CLASSES], f32, space="PSUM")
            for st in states:
                st["rstd"] = rstd
                st["fc"] = fc
            prev_states = states
            prev_gst = {"s": s, "tb": tb, "fc": fc}

        # final group's pass B
        for st in prev_states:
            emit_b1(st)
            emit_b2(st)
        emit_outadd(prev_gst)

        out_view = out_d.rearrange("(t p) c -> p t c", p=P)
        acc_view = out_acc[:].rearrange("p (t c) -> p t c", c=NUM_CLASSES)
        nc.sync.dma_start(out_view, acc_view)

    nc.compile()
    return nc


def _ensure_ntff_hook():
    import sys, types
    try:
        from antenv.axon_hooks import get_axon_ntff_profile_hook  # noqa: F401
        return
    except ImportError:
        pass
    mod = types.ModuleType("antenv.axon_hooks")
    _hook = [None]
    mod.set_axon_ntff_profile_hook = lambda h: _hook.__setitem__(0, h)
    mod.get_axon_ntff_profile_hook = lambda: _hook[0]
    sys.modules["antenv.axon_hooks"] = mod
    try:
        import antenv
        antenv.axon_hooks = mod
    except ImportError:
        pass
    try:
        from trn_agent_boot.trn_boot import _ntff_profile_via_ctypes
        mod.set_axon_ntff_profile_hook(
            _ntff_profile_via_ctypes("/opt/axon/libaxon_pjrt.so"))
    except Exception:
        pass


# ----------------------------------------------------------------------------
# entry point
# ----------------------------------------------------------------------------
def kernel(x, edge_index, edge_weight, W1, b1, ln_g, ln_b, Wfc, bfc):
    global LAST_RESULTS
    from concourse.bass_utils import run_bass_kernel_spmd

    x16 = np.asarray(x, dtype=np.float32).astype(np.float16)
    meta = _preprocess(edge_index, edge_weight)
    IDXC = meta["IDXC"]

    if "prog" not in _PROGRAM_CACHE:
        _PROGRAM_CACHE["prog"] = _build_program()
    nc = _PROGRAM_CACHE["prog"]

    W1f = np.asarray(W1, np.float32)
    W1aug = np.zeros((IN_CH, HIDDEN + 1), dtype=np.float16)
    W1aug[:, :HIDDEN] = W1f.astype(np.float16)
    W1aug[:, HIDDEN] = (-W1f.mean(axis=1)).astype(np.float16)
    b1f = np.asarray(b1, np.float32).reshape(-1)
    c1 = (b1f - b1f.mean()).reshape(1, HIDDEN).astype(np.float32)

    rows = meta["node_tile"] * P + meta["node_slot"]
    xts = np.zeros((P, TILES * P), dtype=np.float16)
    xts[:, rows] = (x16.astype(np.float32)
                    * meta["selfnorm16"].astype(np.float32)[:, None]
                    ).astype(np.float16).T

    banks = {}
    for b in range(NBANK):
        blk = np.zeros((BANK, IN_CH), dtype=np.float16)
        seg = x16[b * BANK:(b + 1) * BANK]
        blk[:len(seg)] = seg
        banks[f"xb{b}"] = blk

    common = dict(
        banks,
        W1aug=W1aug,
        Wfc=np.asarray(Wfc, np.float32).astype(np.float16),
        c1=c1,
        ln_g=np.asarray(ln_g, np.float32).reshape(HIDDEN, 1),
        ln_b=np.asarray(ln_b, np.float32).reshape(HIDDEN, 1),
        bfc=np.asarray(bfc, np.float32).reshape(1, NUM_CLASSES),
        iota=np.arange(P, dtype=np.float16).reshape(1, P),
        idm=np.eye(P, dtype=np.float16),
    )
    MC = TPC * NBANK * K * 2
    in_maps = []
    for core in range(N_CORES):
        msl = slice(core * MC, (core + 1) * MC)
        ssl = slice(core * TPC * P, (core + 1) * TPC * P)
        in_maps.append(dict(
            common,
            idx=np.ascontiguousarray(meta["idx_all"][:, core * IDXC:(core + 1) * IDXC]),
            dstl=np.ascontiguousarray(meta["dstl_all"][:, msl]),
            normv=np.ascontiguousarray(meta["norm_all"][:, msl]),
            xts=np.ascontiguousarray(xts[:, ssl]),
        ))

    trace = bool(os.environ.get("KERNEL_TRACE"))
    if trace:
        _ensure_ntff_hook()
    res = run_bass_kernel_spmd(nc, in_maps, list(range(N_CORES)), trace=trace)
    LAST_RESULTS = res

    all_rows = np.concatenate([res.results[c]["out"] for c in range(N_CORES)],
                              axis=0)
    return np.ascontiguousarray(all_rows[rows].astype(np.float32))


# revision 41
# speedup vs baseline: 1.2081x; 1.0201x over previous
"""GCN classifier (GCNConv + LayerNorm + ReLU + Linear) on 8 Trainium2 NeuronCores.

v2 strategy (self-contained; sized for N=100000, E=1600000, 128 ch, 16 classes):
  out = LN((A @ x) @ W1 + b1).relu() @ Wfc + bfc,  A = normalized adjacency.

  Profiling insights driving this design (vs v1 baseline @ 2.48ms):
  - SWDGE descriptor generation on the Q7 cores is ~8ns/descriptor and was
    2.0ms serial on one core pair. Fix: num_swdge_queues=4, one gather call
    per source bank on its own queue_num -> 4 Q7 pairs generate in parallel.
  - DVE tensor_scalar/copy can enter 2-port perf mode which takes an
    exclusive lock on the SBUF port shared with GPSIMD -> one-hot builds
    were blocking descriptor generation (and vice versa), 4.2ms of DVE time.
    Fix: build one-hot slabs with tensor_tensor (never contends) using
    stride-0 repeat APs; PSUM evacuations / scaling moved to the ACT engine
    (own SBUF port).
  - fp32 matmuls are 4 cycles/row on the PE; fp16 is 1. Everything on the
    matmul path is fp16 (tolerance is 2e-2; fp16 keeps us ~1e-3).
  - Padding trimmed: nodes are packed into 888 tiles so every (tile, bank)
    cell fits exactly K=4 chunks of 128 edges (~7% pad vs ~30%); self-loops
    are not gathered at all - they stream as dense 128-row blocks from a
    tile-permuted fp16 copy of x via HWDGE (free of Q7 descriptor cost).
  - LayerNorm mean comes free as an extra (negated row-mean) column of the
    W1 matmul; LN affine + ReLU fold into one ACT op in transposed layout.
"""
import heapq
import os

import numpy as np

N_NODES = 100000
IN_CH = 128
HIDDEN = 128
NUM_CLASSES = 16
LN_EPS = 1e-5
N_CORES = 8
P = 128
BANK = 25000
NBANK = 4
K = 4                 # chunks per (tile, bank)
CPT = NBANK * K + 1   # chunks per tile (16 gather + 1 dense self block)
CELLCAP = K * P       # max edges per (tile, bank)
TILES = 848
TPC = TILES // N_CORES
GS = 6                # tiles per gather group

LAST_RESULTS = None
_PROGRAM_CACHE = {}


def _groups():
    out = []
    t = 0
    while t < TPC:
        s = min(GS, TPC - t)
        out.append((s, t))
        t += s
    return out


def _call_col_bases():
    """Column base (in 16-wide int16 idx columns) of each (group, bank) gather."""
    bases = []
    run = 0
    for s, _ in _groups():
        row = []
        for _b in range(NBANK):
            row.append(run)
            run += s * K * P // 16
        bases.append(row)
    return bases, run


# ----------------------------------------------------------------------------
# host-side preprocessing
# ----------------------------------------------------------------------------
def _assign_tiles(dst, eb, cnt_nb):
    """LPT-pack nodes into TILES tiles (<=128 nodes each), then repair so every
    (tile, bank) cell holds <= CELLCAP edges."""
    N = N_NODES
    cnt = cnt_nb.sum(axis=1)
    order = np.argsort(-cnt, kind="stable")
    heap = [(0, t) for t in range(TILES)]
    heapq.heapify(heap)
    node_cnt = np.zeros(TILES, dtype=np.int64)
    edge_sum = np.zeros(TILES, dtype=np.int64)
    node_tile = np.empty(N, dtype=np.int64)
    for nd in order:
        while True:
            s, t = heapq.heappop(heap)
            if node_cnt[t] < P:
                break
        node_tile[nd] = t
        node_cnt[t] += 1
        edge_sum[t] += cnt[nd]
        if node_cnt[t] < P:
            heapq.heappush(heap, (edge_sum[t], t))

    # repair per-bank overflows
    for _ in range(64):
        cell = np.zeros((TILES, NBANK), dtype=np.int64)
        np.add.at(cell, (node_tile[dst], eb), 1)
        over = np.argwhere(cell > CELLCAP)
        if len(over) == 0:
            break
        node_cnt = np.bincount(node_tile, minlength=TILES)
        for t, b in over:
            excess = cell[t, b] - CELLCAP
            if excess <= 0:
                continue
            nodes_t = np.where(node_tile == t)[0]
            cand = nodes_t[np.argsort(-cnt_nb[nodes_t, b], kind="stable")]
            for nd in cand:
                if excess <= 0:
                    break
                c_nd = cnt_nb[nd]
                if c_nd[b] == 0:
                    break
                ok = (node_cnt < P) & ((cell + c_nd[None, :]) <= CELLCAP).all(axis=1)
                ok[t] = False
                if not ok.any():
                    continue
                cand_t2 = np.where(ok)[0]
                t2 = cand_t2[np.argmin(cell[cand_t2].sum(axis=1))]
                node_tile[nd] = t2
                cell[t] -= c_nd
                cell[t2] += c_nd
                node_cnt[t] -= 1
                node_cnt[t2] += 1
                excess = cell[t, b] - CELLCAP
    else:
        raise RuntimeError("tile repair did not converge")

    # compact slots within each tile
    order2 = np.argsort(node_tile, kind="stable")
    tile_sorted = node_tile[order2]
    starts = np.zeros(TILES + 1, dtype=np.int64)
    np.cumsum(np.bincount(tile_sorted, minlength=TILES), out=starts[1:])
    node_slot = np.empty(N, dtype=np.int64)
    node_slot[order2] = np.arange(N) - starts[tile_sorted]
    assert (node_slot < P).all()
    return node_tile, node_slot


def _preprocess(edge_index, edge_weight):
    src = np.asarray(edge_index[0], dtype=np.int64)
    dst = np.asarray(edge_index[1], dtype=np.int64)
    w = np.asarray(edge_weight, dtype=np.float32)
    N = N_NODES

    deg = np.bincount(dst, weights=w.astype(np.float64), minlength=N) + 1.0
    dinv = (1.0 / np.sqrt(deg)).astype(np.float32)
    norm = (dinv[src] * w * dinv[dst]).astype(np.float32)
    selfnorm = (dinv.astype(np.float64) ** 2).astype(np.float32)  # 1/deg

    eb = src // BANK
    cnt_nb = np.zeros((N, NBANK), dtype=np.int64)
    np.add.at(cnt_nb, (dst, eb), 1)
    node_tile, node_slot = _assign_tiles(dst, eb, cnt_nb)

    # per-edge (tile, bank) cell position
    et = node_tile[dst]
    keys = et * NBANK + eb
    eorder = np.argsort(keys, kind="stable")
    keys_s = keys[eorder]
    cum = np.zeros(TILES * NBANK + 1, dtype=np.int64)
    np.cumsum(np.bincount(keys_s, minlength=TILES * NBANK), out=cum[1:])
    pos = np.arange(len(keys_s)) - cum[keys_s]
    kk = pos // P
    lane = pos % P
    assert (kk < K).all(), "cell overflow after repair"

    src_s = src[eorder]
    dst_s = dst[eorder]
    et_s = et[eorder]
    eb_s = eb[eorder]
    norm_s = norm[eorder]

    # tile-major metadata, duplicated-pair layout [128, tile*(16*2) + (bank*K+kk)*2 + {0,1}]
    # (pairs give every DVE operand an innermost stride-1 dim -> 2x perf mode)
    GCH = NBANK * K  # gathered chunks per tile (self handled via selfoh)
    MCOLS = TILES * GCH * 2
    mcol = (et_s * GCH + eb_s * K + kk) * 2
    dstl_all = np.zeros((P, MCOLS), dtype=np.float16)
    norm_all = np.zeros((P, MCOLS), dtype=np.float16)
    dstl_all[lane, mcol] = node_slot[dst_s].astype(np.float16)
    dstl_all[lane, mcol + 1] = dstl_all[lane, mcol]
    norm_all[lane, mcol] = norm_s.astype(np.float16)
    norm_all[lane, mcol + 1] = norm_all[lane, mcol]

    # self-loop contribution folded post-aggregation: transposed, pre-scaled
    # x rows in tile-permuted order; added into Ps [ch, d] by one DVE op/tile
    perm_rows = node_tile * P + node_slot
    selfnorm16 = selfnorm.astype(np.float16)

    # gather indices, call-major: per core, per (group, bank) call,
    # within call linear i = (j*K + kk)*128 + lane
    bases, IDXC = _call_col_bases()
    core = et_s // TPC
    tl = et_s % TPC
    g = tl // GS
    j = tl % GS
    cb = np.asarray([[bases[gi][bi] for bi in range(NBANK)]
                     for gi in range(len(bases))], dtype=np.int64)
    i_lin = (j * K + kk) * P + lane
    col16 = core * IDXC + cb[g, eb_s] + i_lin // 16
    row16 = i_lin % 16
    idx16 = np.zeros((16, N_CORES * IDXC), dtype=np.int16)
    idx16[row16, col16] = (src_s % BANK).astype(np.int16)
    idx_all = np.tile(idx16, (8, 1))

    return dict(
        idx_all=idx_all, norm_all=norm_all, dstl_all=dstl_all,
        perm_rows=perm_rows, selfnorm16=selfnorm16,
        node_tile=node_tile, node_slot=node_slot, IDXC=IDXC,
    )


# ----------------------------------------------------------------------------
# device program
# ----------------------------------------------------------------------------
def _build_program():
    from contextlib import ExitStack
    import concourse.bass as bass
    import concourse.tile as tile
    from concourse import bacc, mybir

    f32 = mybir.dt.float32
    f16 = mybir.dt.float16
    i16 = mybir.dt.int16
    H = HIDDEN
    GCH = NBANK * K
    MC = TPC * GCH * 2
    bases, IDXC = _call_col_bases()

    nc = bacc.Bacc("TRN2", target_bir_lowering=False, debug=False,
                   num_devices=N_CORES, num_swdge_queues=4)
    xb = [nc.dram_tensor(f"xb{b}", [BANK, IN_CH], f16, kind="ExternalInput").ap()
          for b in range(NBANK)]
    xts_d = nc.dram_tensor("xts", [P, TPC * P], f16, kind="ExternalInput").ap()
    idx_d = nc.dram_tensor("idx", [P, IDXC], i16, kind="ExternalInput").ap()
    dstl_d = nc.dram_tensor("dstl", [P, MC], f16, kind="ExternalInput").ap()
    norm_d = nc.dram_tensor("normv", [P, MC], f16, kind="ExternalInput").ap()
    w1_d = nc.dram_tensor("W1aug", [IN_CH, H + 1], f16, kind="ExternalInput").ap()
    wfc_d = nc.dram_tensor("Wfc", [H, NUM_CLASSES], f16, kind="ExternalInput").ap()
    c1_d = nc.dram_tensor("c1", [1, H], f32, kind="ExternalInput").ap()
    lng_d = nc.dram_tensor("ln_g", [H, 1], f32, kind="ExternalInput").ap()
    lnb_d = nc.dram_tensor("ln_b", [H, 1], f32, kind="ExternalInput").ap()
    bfc_d = nc.dram_tensor("bfc", [1, NUM_CLASSES], f32, kind="ExternalInput").ap()
    iota_d = nc.dram_tensor("iota", [1, P], f16, kind="ExternalInput").ap()
    idm_d = nc.dram_tensor("idm", [P, P], f16, kind="ExternalInput").ap()
    out_d = nc.dram_tensor("out", [TPC * P, NUM_CLASSES], f32,
                           kind="ExternalOutput").ap()

    def bcast(src_ap, parts=P):
        return bass.AP(tensor=src_ap.tensor, offset=src_ap.offset,
                       ap=[[0, parts]] + list(src_ap.ap[1:]))

    def rep_mid(ap2d, n):
        """[p, q] -> [p, n, q] with the middle dim broadcast (stride 0)."""
        a = list(ap2d.ap)
        return bass.AP(tensor=ap2d.tensor, offset=ap2d.offset,
                       ap=[a[0], [0, n], a[1]])

    # 4D APs for the one-hot slab build; every operand keeps an innermost
    # stride-1 dim of size 2 so the DVE can enter 2x_1P perf mode.
    def meta_rep4(ap2d):
        """paired meta [p, 2*GCH] -> [p, GCH, 64, 2]; value const along dim 64."""
        a = list(ap2d.ap)
        return bass.AP(tensor=ap2d.tensor, offset=ap2d.offset,
                       ap=[a[0], [2, GCH], [0, P // 2], [1, 2]])

    def iota_rep4(ap2d):
        """IOTA [p, 128] -> [p, GCH, 64, 2]; iota along the last two dims."""
        a = list(ap2d.ap)
        return bass.AP(tensor=ap2d.tensor, offset=ap2d.offset,
                       ap=[a[0], [0, GCH], [2, P // 2], [1, 2]])

    def slab4(ap2d):
        """slab [p, GCH*128] -> [p, GCH, 64, 2] contiguous."""
        a = list(ap2d.ap)
        return bass.AP(tensor=ap2d.tensor, offset=ap2d.offset,
                       ap=[a[0], [P, GCH], [2, P // 2], [1, 2]])

    AL = mybir.AluOpType
    AF = mybir.ActivationFunctionType

    with tile.TileContext(nc) as tc, ExitStack() as ctx:
        consts = ctx.enter_context(tc.tile_pool(name="consts", bufs=1))
        gpool = ctx.enter_context(tc.tile_pool(name="gather", bufs=4))
        ohp = ctx.enter_context(tc.tile_pool(name="onehot", bufs=3))
        sp = ctx.enter_context(tc.tile_pool(name="work", bufs=4))
        t1p = ctx.enter_context(tc.tile_pool(name="t1c", bufs=2 * GS))
        statp = ctx.enter_context(tc.tile_pool(name="stats", bufs=4 * GS))
        pp_ps = ctx.enter_context(tc.tile_pool(name="pp_ps", bufs=2, space="PSUM"))
        agg_ps = ctx.enter_context(tc.tile_pool(name="agg_ps", bufs=2, space="PSUM"))
        tr_ps = ctx.enter_context(tc.tile_pool(name="tr_ps", bufs=2, space="PSUM"))
        fc_ps = ctx.enter_context(tc.tile_pool(name="fc_ps", bufs=2, space="PSUM"))

        W1s = consts.tile([IN_CH, H + 1], f16)
        nc.sync.dma_start(W1s[:], w1_d[:])
        Wfcs = consts.tile([H, NUM_CLASSES], f16)
        nc.sync.dma_start(Wfcs[:], wfc_d[:])
        C1s = consts.tile([P, H], f32)
        nc.sync.dma_start(C1s[:], bcast(c1_d))
        Gcol = consts.tile([H, 1], f32)
        nc.sync.dma_start(Gcol[:], lng_d[:])
        Bcol = consts.tile([H, 1], f32)
        nc.sync.dma_start(Bcol[:], lnb_d[:])
        BFCs = consts.tile([P, NUM_CLASSES], f32)
        nc.sync.dma_start(BFCs[:], bcast(bfc_d))
        IOTA = consts.tile([P, P], f16)
        nc.sync.dma_start(IOTA[:], bcast(iota_d))
        idents = consts.tile([P, P], f16)
        nc.sync.dma_start(idents[:], idm_d[:])
        eps_t = consts.tile([P, 1], f32)
        nc.vector.memset(eps_t[:], LN_EPS)

        # metadata tiles loaded in per-group slices inside the loop so group 0
        # starts as soon as its own slice lands
        idx_s = consts.tile([P, IDXC], i16)
        dstl_s = consts.tile([P, MC], f16)
        norm_s = consts.tile([P, MC], f16)

        out_acc = consts.tile([P, TPC * NUM_CLASSES], f32)

        # Software-pipelined emission: the PE stream is kept dense by skewing
        # every cross-engine round trip behind enough chunk-matmul work that
        # its dependencies are already resolved when the PE (or DVE/ACT FIFO
        # head) reaches it.
        def emit_add(st):
            # fold self-loop contribution: Ps2 = Ps + (selfnorm-scaled x^T)
            Ps2 = sp.tile([IN_CH, P], f16, tag="Ps2")
            j = st["j"]
            nc.vector.tensor_tensor(
                out=Ps2[:], in0=st["Ps"][:],
                in1=st["xtg"][:, j * P:(j + 1) * P], op=AL.add)
            st["Ps2"] = Ps2

        def emit_agg(st):
            agg = agg_ps.tile([P, H + 1], f32, space="PSUM")
            nc.tensor.matmul(agg[:], lhsT=st["Ps2"][:], rhs=W1s[:],
                             start=True, stop=True)
            st["agg"] = agg

        def emit_stats(st):
            agg = st["agg"]
            mu = statp.tile([P, 1], f32, tag="mu")
            nc.scalar.activation(out=mu[:], in_=agg[:, H:H + 1], func=AF.Copy)
            t1c = t1p.tile([P, H], f32, tag="t1c")
            nc.vector.scalar_tensor_tensor(
                out=t1c[:], in0=agg[:, 0:H], scalar=mu[:], in1=C1s[:],
                op0=AL.add, op1=AL.add)
            sq = sp.tile([P, H], f32, tag="sq")
            nc.scalar.activation(out=sq[:], in_=t1c[:], func=AF.Square,
                                 accum_out=st["vars"][:, st["j"]:st["j"] + 1])
            st["t1c"] = t1c

        def emit_b1(st):
            t1n = sp.tile([P, H], f16, tag="t1n")
            nc.scalar.activation(out=t1n[:], in_=st["t1c"][:], func=AF.Copy,
                                 scale=st["rstd"][:, st["j"]:st["j"] + 1])
            yT = tr_ps.tile([H, P], f16, space="PSUM")
            nc.tensor.transpose(out=yT[:], in_=t1n[:], identity=idents[:])
            st["yT"] = yT

        def emit_b2(st):
            hrT = sp.tile([H, P], f16, tag="hrT")
            nc.scalar.activation(out=hrT[:], in_=st["yT"][:], func=AF.Relu,
                                 scale=Gcol[:], bias=Bcol[:])
            j = st["j"]
            nc.tensor.matmul(st["fc"][:, j * NUM_CLASSES:(j + 1) * NUM_CLASSES],
                             lhsT=hrT[:], rhs=Wfcs[:], start=True, stop=True)

        def emit_outadd(gst):
            s, tb, fc = gst["s"], gst["tb"], gst["fc"]
            oslice = out_acc[:, tb * NUM_CLASSES:(tb + s) * NUM_CLASSES]
            o3 = oslice.rearrange("p (t c) -> p t c", c=NUM_CLASSES)
            f3 = fc[:, 0:s * NUM_CLASSES].rearrange("p (t c) -> p t c",
                                                    c=NUM_CLASSES)
            nc.vector.tensor_tensor(out=o3, in0=f3,
                                    in1=rep_mid(BFCs[:], s), op=AL.add)

        prev_states = None   # tile states of the previous group (pass B pending)
        prev_gst = None
        for gi, (s, tb) in enumerate(_groups()):
            ic0 = bases[gi][0]
            ic1 = bases[gi][NBANK - 1] + s * K * P // 16
            nc.sync.dma_start(idx_s[:, ic0:ic1], idx_d[:, ic0:ic1])
            mc_lo = tb * GCH * 2
            mc_hi = (tb + s) * GCH * 2
            nc.sync.dma_start(dstl_s[:, mc_lo:mc_hi], dstl_d[:, mc_lo:mc_hi])
            nc.sync.dma_start(norm_s[:, mc_lo:mc_hi], norm_d[:, mc_lo:mc_hi])

            Gg = gpool.tile([P, s * GCH, IN_CH], f16, tag="Gg")
            for b in range(NBANK):
                n = s * K * P
                cbase = bases[gi][b]
                nc.gpsimd.dma_gather(
                    out_ap=Gg[:, b * s * K:(b + 1) * s * K, :],
                    in_ap=xb[b][:],
                    idxs_ap=idx_s[:, cbase:cbase + n // 16],
                    num_idxs=n, num_idxs_reg=n, elem_size=IN_CH,
                    single_packet=False, queue_num=b,
                )
            # self-loop columns (pre-scaled x^T, tile-permuted) via HWDGE
            xtg = gpool.tile([P, s * P], f16, tag="xtg")
            xt_in = bass.AP(tensor=xts_d.tensor,
                            offset=xts_d.offset + tb * P,
                            ap=[[TPC * P, P], [1, s * P]])
            nc.sync.dma_start(xtg[:], xt_in)

            vars_g = statp.tile([P, s], f32, tag="vars")
            states = []
            for j in range(s):
                t = tb + j
                mc0 = t * GCH * 2
                dsl = dstl_s[:, mc0:mc0 + GCH * 2]
                nsl = norm_s[:, mc0:mc0 + GCH * 2]
                tmp = ohp.tile([P, GCH * P], f16, tag="tmp")
                nc.vector.tensor_tensor(out=slab4(tmp[:]), in0=meta_rep4(dsl),
                                        in1=iota_rep4(IOTA[:]), op=AL.is_equal)
                ohs = ohp.tile([P, GCH * P], f16, tag="ohs")
                nc.vector.tensor_tensor(out=slab4(ohs[:]), in0=slab4(tmp[:]),
                                        in1=meta_rep4(nsl), op=AL.mult)

                Pp = pp_ps.tile([IN_CH, P], f32, space="PSUM")
                for c in range(GCH):
                    gcol = (c // K) * s * K + j * K + (c % K)
                    nc.tensor.matmul(Pp[:], lhsT=Gg[:, gcol, :],
                                     rhs=ohs[:, c * P:(c + 1) * P],
                                     start=(c == 0), stop=(c == GCH - 1))
                Ps = sp.tile([IN_CH, P], f16, tag="Ps")
                nc.scalar.activation(out=Ps[:], in_=Pp[:], func=AF.Copy)
                states.append({"j": j, "Ps": Ps, "vars": vars_g, "xtg": xtg})

                # skewed tails: add+agg one tile behind, stats two tiles behind
                if j >= 1:
                    emit_add(states[j - 1])
                    emit_agg(states[j - 1])
                if j >= 2:
                    emit_stats(states[j - 2])
                # previous group's pass B: b1 at tile j, b2 one tile later
                if prev_states is not None:
                    if j < len(prev_states):
                        emit_b1(prev_states[j])
                    if 1 <= j <= len(prev_states):
                        emit_b2(prev_states[j - 1])

            # group-boundary flush of pass A
            emit_add(states[s - 1])
            emit_agg(states[s - 1])
            for jj in range(max(0, s - 2), s):
                emit_stats(states[jj])
            if prev_states is not None:
                for k in range(s, len(prev_states)):
                    emit_b1(prev_states[k])
                for k in range(max(1, s), len(prev_states) + 1):
                    emit_b2(prev_states[k - 1])
                emit_outadd(prev_gst)

            stdg = statp.tile([P, s], f32, tag="std")
            nc.scalar.activation(out=stdg[:], in_=vars_g[:], func=AF.Sqrt,
                                 bias=eps_t[:], scale=1.0 / H)
            rstd = statp.tile([P, s], f32, tag="rstd")
            nc.vector.reciprocal(out=rstd[:], in_=stdg[:])
            fc = fc_ps.tile([P, s * NUM_CLASSES], f32, space="PSUM")
            for st in states:
                st["rstd"] = rstd
                st["fc"] = fc
            prev_states = states
            prev_gst = {"s": s, "tb": tb, "fc": fc}

        # final group's pass B
        for st in prev_states:
            emit_b1(st)
            emit_b2(st)
        emit_outadd(prev_gst)

        out_view = out_d.rearrange("(t p) c -> p t c", p=P)
        acc_view = out_acc[:].rearrange("p (t c) -> p t c", c=NUM_CLASSES)
        nc.sync.dma_start(out_view, acc_view)

    nc.compile()
    return nc


def _ensure_ntff_hook():
    import sys, types
    try:
        from antenv.axon_hooks import get_axon_ntff_profile_hook  # noqa: F401
        return
    except ImportError:
        pass
    mod = types.ModuleType("antenv.axon_hooks")
    _hook = [None]
    mod.set_axon_ntff_profile_hook = lambda h: _hook.__setitem__(0, h)
    mod.get_axon_ntff_profile_hook = lambda: _hook[0]
    sys.modules["antenv.axon_hooks"] = mod
    try:
        import antenv
        antenv.axon_hooks = mod
    except ImportError:
        pass
    try:
        from trn_agent_boot.trn_boot import _ntff_profile_via_ctypes
        mod.set_axon_ntff_profile_hook(
            _ntff_profile_via_ctypes("/opt/axon/libaxon_pjrt.so"))
    except Exception:
        pass


# ----------------------------------------------------------------------------
# entry point
# ----------------------------------------------------------------------------
def kernel(x, edge_index, edge_weight, W1, b1, ln_g, ln_b, Wfc, bfc):
    global LAST_RESULTS
    from concourse.bass_utils import run_bass_kernel_spmd

    x16 = np.asarray(x, dtype=np.float32).astype(np.float16)
    meta = _preprocess(edge_index, edge_weight)
    IDXC = meta["IDXC"]

    if "prog" not in _PROGRAM_CACHE:
        _PROGRAM_CACHE["prog"] = _build_program()
    nc = _PROGRAM_CACHE["prog"]

    W1f = np.asarray(W1, np.float32)
    W1aug = np.zeros((IN_CH, HIDDEN + 1), dtype=np.float16)
    W1aug[:, :HIDDEN] = W1f.astype(np.float16)
    W1aug[:, HIDDEN] = (-W1f.mean(axis=1)).astype(np.float16)
    b1f = np.asarray(b1, np.float32).reshape(-1)
    c1 = (b1f - b1f.mean()).reshape(1, HIDDEN).astype(np.float32)

    rows = meta["node_tile"] * P + meta["node_slot"]
    xts = np.zeros((P, TILES * P), dtype=np.float16)
    xts[:, rows] = (x16.astype(np.float32)
                    * meta["selfnorm16"].astype(np.float32)[:, None]
                    ).astype(np.float16).T

    banks = {}
    for b in range(NBANK):
        blk = np.zeros((BANK, IN_CH), dtype=np.float16)
        seg = x16[b * BANK:(b + 1) * BANK]
        blk[:len(seg)] = seg
        banks[f"xb{b}"] = blk

    common = dict(
        banks,
        W1aug=W1aug,
        Wfc=np.asarray(Wfc, np.float32).astype(np.float16),
        c1=c1,
        ln_g=np.asarray(ln_g, np.float32).reshape(HIDDEN, 1),
        ln_b=np.asarray(ln_b, np.float32).reshape(HIDDEN, 1),
        bfc=np.asarray(bfc, np.float32).reshape(1, NUM_CLASSES),
        iota=np.arange(P, dtype=np.float16).reshape(1, P),
        idm=np.eye(P, dtype=np.float16),
    )
    MC = TPC * NBANK * K * 2
    in_maps = []
    for core in range(N_CORES):
        msl = slice(core * MC, (core + 1) * MC)
        ssl = slice(core * TPC * P, (core + 1) * TPC * P)
        in_maps.append(dict(
            common,
            idx=np.ascontiguousarray(meta["idx_all"][:, core * IDXC:(core + 1) * IDXC]),
            dstl=np.ascontiguousarray(meta["dstl_all"][:, msl]),
            normv=np.ascontiguousarray(meta["norm_all"][:, msl]),
            xts=np.ascontiguousarray(xts[:, ssl]),
        ))

    trace = bool(os.environ.get("KERNEL_TRACE"))
    if trace:
        _ensure_ntff_hook()
    res = run_bass_kernel_spmd(nc, in_maps, list(range(N_CORES)), trace=trace)
    LAST_RESULTS = res

    all_rows = np.concatenate([res.results[c]["out"] for c in range(N_CORES)],
                              axis=0)
    return np.ascontiguousarray(all_rows[rows].astype(np.float32))
